# revision 7
# baseline (speedup 1.0000x reference)
"""Bass/Trainium2 kernel for nn_BilinearSampler (topk_masking).

Contract: kernel(**inputs) takes FULL numpy inputs
  feature_maps [4,256,512,512] f32, sample_points [4,4096,2] f32,
  mask_scores [4,2,512,512] f32
and returns the full reference output tuple:
  (sampled_features_target [4,4096,256], point [4,4096,2],
   sampled_features_source [4,4096,256])

Sharding: 8 cores = 4 batches x 2 halves of the 4096 points. Each core
holds its batch's feature map in site-major [H*W, D] layout so every
bilinear corner pair is one contiguous 2KiB read, served by indirect
DMA gathers (one row-pair per point per descriptor).
"""

import sys

import numpy as np

try:
    import concourse.bass as bass
except ImportError:  # pragma: no cover
    sys.path.insert(0, "/opt/trn_rl_repo")
    import concourse.bass as bass

import concourse.bacc as bacc
import concourse.mybir as mybir
import concourse.tile as tile
from concourse.bass_utils import run_bass_kernel_spmd

P = 128
NPTS = 2048              # points per core
NCH = NPTS // P          # 16 chunks
H = W = 512
D = 256
S = 520                  # padded mask row stride (and row count)
MP_FLAT = 128 * 2115     # 270720 >= 520*520, 128-partition padded
NEG = -1.0e30

f32 = mybir.dt.float32
i32 = mybir.dt.int32
Alu = mybir.AluOpType
Ax = mybir.AxisListType

_CACHE = {}


def _exact_floor(nc, pool, v, name):
    """floor(v) for f32 tile v (any rounding mode on the f32->i32 cast).

    Returns (floor_f32_tile). cast is round-nearest-even on HW; fix by
    subtracting 1 wherever cast(v) > v.
    """
    shp = list(v.shape)
    ti = pool.tile(shp, i32, tag=f"{name}_i")
    nc.vector.tensor_copy(ti[:], v[:])
    tf = pool.tile(shp, f32, tag=f"{name}_f")
    nc.vector.tensor_copy(tf[:], ti[:])
    gt = pool.tile(shp, f32, tag=f"{name}_gt")
    nc.vector.tensor_tensor(out=gt[:], in0=tf[:], in1=v[:], op=Alu.is_gt)
    fl = pool.tile(shp, f32, tag=f"{name}_fl")
    nc.vector.tensor_tensor(out=fl[:], in0=tf[:], in1=gt[:], op=Alu.subtract)
    return fl


def _grid_to_pix(nc, pool, coord, scale, name):
    """Mirror reference float ops: g = coord/256 - 1; pix = ((g+1)*scale-1)*0.5."""
    g = pool.tile(list(coord.shape), f32, tag=f"{name}_g")
    nc.vector.tensor_scalar(
        out=g[:], in0=coord[:], scalar1=1.0 / 256.0, scalar2=-1.0,
        op0=Alu.mult, op1=Alu.add,
    )
    pix = pool.tile(list(coord.shape), f32, tag=f"{name}_pix")
    # (g + 1) * scale
    nc.vector.tensor_scalar(
        out=pix[:], in0=g[:], scalar1=1.0, scalar2=float(scale),
        op0=Alu.add, op1=Alu.mult,
    )
    # (x - 1) * 0.5
    nc.vector.tensor_scalar(
        out=pix[:], in0=pix[:], scalar1=-1.0, scalar2=0.5,
        op0=Alu.add, op1=Alu.mult,
    )
    return pix


def _axis_slot_weights(nc, pool, x0f, fr1, lim, name):
    """Fetch-base and slot weights along one axis.

    x0f: floor pixel coord (f32), fr1 = frac (weight of corner x0+1).
    Returns (xs f32 clamped fetch base in [0, lim-2], a0, a1 slot weights).
    """
    shp = list(x0f.shape)
    xs = pool.tile(shp, f32, tag=f"{name}_xs")
    nc.vector.tensor_scalar(
        out=xs[:], in0=x0f[:], scalar1=0.0, scalar2=float(lim - 2),
        op0=Alu.max, op1=Alu.min,
    )
    d = pool.tile(shp, f32, tag=f"{name}_d")
    nc.vector.tensor_tensor(out=d[:], in0=x0f[:], in1=xs[:], op=Alu.subtract)
    e0 = pool.tile(shp, f32, tag=f"{name}_e0")
    nc.vector.tensor_scalar(out=e0[:], in0=d[:], scalar1=0.0, scalar2=None, op0=Alu.is_equal)
    em = pool.tile(shp, f32, tag=f"{name}_em")
    nc.vector.tensor_scalar(out=em[:], in0=d[:], scalar1=-1.0, scalar2=None, op0=Alu.is_equal)
    ep = pool.tile(shp, f32, tag=f"{name}_ep")
    nc.vector.tensor_scalar(out=ep[:], in0=d[:], scalar1=1.0, scalar2=None, op0=Alu.is_equal)
    fr0 = pool.tile(shp, f32, tag=f"{name}_fr0")
    nc.vector.tensor_scalar(
        out=fr0[:], in0=fr1[:], scalar1=-1.0, scalar2=1.0, op0=Alu.mult, op1=Alu.add
    )
    # a0 = fr0*e0 + fr1*em ; a1 = fr1*e0 + fr0*ep
    a0 = pool.tile(shp, f32, tag=f"{name}_a0")
    nc.vector.tensor_tensor(out=a0[:], in0=fr0[:], in1=e0[:], op=Alu.mult)
    t = pool.tile(shp, f32, tag=f"{name}_t")
    nc.vector.tensor_tensor(out=t[:], in0=fr1[:], in1=em[:], op=Alu.mult)
    nc.vector.tensor_tensor(out=a0[:], in0=a0[:], in1=t[:], op=Alu.add)
    a1 = pool.tile(shp, f32, tag=f"{name}_a1")
    nc.vector.tensor_tensor(out=a1[:], in0=fr1[:], in1=e0[:], op=Alu.mult)
    nc.vector.tensor_tensor(out=t[:], in0=fr0[:], in1=ep[:], op=Alu.mult)
    nc.vector.tensor_tensor(out=a1[:], in0=a1[:], in1=t[:], op=Alu.add)
    return xs, a0, a1


def _bilinear_setup(nc, pool, gx, gy, name):
    """From pixel-space point coords (gx->W axis, gy->H axis rows) compute
    per-point gather sites (int32 [P,NCH,2]) and 4 slot weights [P,NCH]."""
    ix = _grid_to_pix(nc, pool, gx, W, f"{name}_ix")
    iy = _grid_to_pix(nc, pool, gy, H, f"{name}_iy")
    x0 = _exact_floor(nc, pool, ix, f"{name}_x0")
    y0 = _exact_floor(nc, pool, iy, f"{name}_y0")
    fx1 = pool.tile([P, NCH], f32, tag=f"{name}_fx1")
    nc.vector.tensor_tensor(out=fx1[:], in0=ix[:], in1=x0[:], op=Alu.subtract)
    fy1 = pool.tile([P, NCH], f32, tag=f"{name}_fy1")
    nc.vector.tensor_tensor(out=fy1[:], in0=iy[:], in1=y0[:], op=Alu.subtract)

    xs, a0, a1 = _axis_slot_weights(nc, pool, x0, fx1, W, f"{name}_x")
    ys, b0, b1 = _axis_slot_weights(nc, pool, y0, fy1, H, f"{name}_y")

    # site0 = ys*512 + xs ; site1 = site0 + 512
    sidx = pool.tile([P, NCH, 2], f32, tag=f"{name}_sidx")
    nc.vector.scalar_tensor_tensor(
        out=sidx[:, :, 0], in0=ys[:], scalar=float(W), in1=xs[:],
        op0=Alu.mult, op1=Alu.add,
    )
    nc.vector.tensor_scalar(
        out=sidx[:, :, 1], in0=sidx[:, :, 0], scalar1=float(W), scalar2=None, op0=Alu.add
    )
    sidx_i = pool.tile([P, NCH, 2], i32, tag=f"{name}_sidxi")
    nc.vector.tensor_copy(sidx_i[:], sidx[:])

    # w00 = a0*b0, w01 = a1*b0, w10 = a0*b1, w11 = a1*b1
    ws = []
    for wi, (ca, cb) in enumerate(((a0, b0), (a1, b0), (a0, b1), (a1, b1))):
        wt = pool.tile([P, NCH], f32, tag=f"{name}_w{wi}")
        nc.vector.tensor_tensor(out=wt[:], in0=ca[:], in1=cb[:], op=Alu.mult)
        ws.append(wt)
    return sidx_i, ws


def build_module():
    nc = bacc.Bacc("TRN2", target_bir_lowering=False, debug=False, num_devices=8)

    feat = nc.dram_tensor("feat", [H * W, D], f32, kind="ExternalInput")
    mask = nc.dram_tensor("mask", [2, H, W], f32, kind="ExternalInput")
    px_in = nc.dram_tensor("px", [NPTS], f32, kind="ExternalInput")
    py_in = nc.dram_tensor("py", [NPTS], f32, kind="ExternalInput")
    wv_in = nc.dram_tensor("withinv", [P, 1, 16], f32, kind="ExternalInput")
    jc_in = nc.dram_tensor("sixteen_minus_j", [P, 1, 16], f32, kind="ExternalInput")
    out_t = nc.dram_tensor("out_t", [NPTS, D], f32, kind="ExternalOutput")
    out_s = nc.dram_tensor("out_s", [NPTS, D], f32, kind="ExternalOutput")
    out_p = nc.dram_tensor("out_p", [NPTS, 2], f32, kind="ExternalOutput")
    mp = nc.dram_tensor("mp", [MP_FLAT, 1], f32)

    mp_h = mp[:].tensor

    with tile.TileContext(nc) as tc:
        with (
            tc.tile_pool(name="consts", bufs=1) as cpool,
            tc.tile_pool(name="scal", bufs=1) as sp,
            tc.tile_pool(name="maskp", bufs=1) as mkp,
            tc.tile_pool(name="span", bufs=3) as spanp,
            tc.tile_pool(name="patch", bufs=4) as patp,
            tc.tile_pool(name="fout", bufs=4) as fp,
        ):
            # ---------------- padded mask build (mp) ----------------
            # fill whole mp with NEG
            negt = cpool.tile([P, 2115], f32)
            nc.vector.memset(negt[:], NEG)
            mp_flat_ap = bass.AP(mp_h, 0, [[2115, P], [1, 2115]])
            fill = nc.sync.dma_start(out=mp_flat_ap, in_=negt[:])

            # load mask [2,512,512], partition p = h % 128 (h = q*128 + p)
            mload = mkp.tile([P, 2, 4, W], f32)
            mask_ap = bass.AP(mask[:].tensor, 0, [[W, P], [H * W, 2], [P * W, 4], [1, W]])
            nc.sync.dma_start(out=mload[:], in_=mask_ap)
            msum = mkp.tile([P, 4, W], f32)
            nc.vector.tensor_tensor(
                out=msum[:], in0=mload[:, 0], in1=mload[:, 1], op=Alu.add
            )
            # interior write: mp[2 + q*128 + p, 2 + w]
            mp_int_ap = bass.AP(mp_h, 2 * S + 2, [[S, P], [P * S, 4], [1, W]])
            intw = nc.sync.dma_start(out=mp_int_ap, in_=msum[:])
            tile.add_dep_helper(intw.ins, fill.ins, reason="mp fill before interior")

            # ---------------- load constants & points ----------------
            wv = cpool.tile([P, 1, 16], f32)
            nc.sync.dma_start(out=wv[:], in_=wv_in[:])
            jc = cpool.tile([P, 1, 16], f32)
            nc.sync.dma_start(out=jc[:], in_=jc_in[:])

            x = sp.tile([P, NCH], f32, tag="x")
            nc.sync.dma_start(out=x[:], in_=px_in[:].rearrange("(c p) -> p c", p=P))
            y = sp.tile([P, NCH], f32, tag="y")
            nc.sync.dma_start(out=y[:], in_=py_in[:].rearrange("(c p) -> p c", p=P))

            # ---------------- source grid setup + gather/accum ----------------
            sidx_s, ws_s = _bilinear_setup(nc, sp, x, y, "src")

            def feat_chunk(c, sidx_i, ws, out_dram, label):
                patch = patp.tile([P, 1024], f32, tag="patch")
                nc.gpsimd.indirect_dma_start(
                    out=patch[:, 0:512], out_offset=None, in_=feat[:],
                    in_offset=bass.IndirectOffsetOnAxis(ap=sidx_i[:, c, 0:1], axis=0),
                )
                nc.gpsimd.indirect_dma_start(
                    out=patch[:, 512:1024], out_offset=None, in_=feat[:],
                    in_offset=bass.IndirectOffsetOnAxis(ap=sidx_i[:, c, 1:2], axis=0),
                )
                f = fp.tile([P, D], f32, tag="f")
                nc.vector.tensor_scalar(
                    out=f[:], in0=patch[:, 0:D], scalar1=ws[0][:, c : c + 1],
                    scalar2=None, op0=Alu.mult,
                )
                for wi, off in ((1, D), (2, 2 * D), (3, 3 * D)):
                    nc.vector.scalar_tensor_tensor(
                        out=f[:], in0=patch[:, off : off + D],
                        scalar=ws[wi][:, c : c + 1], in1=f[:],
                        op0=Alu.mult, op1=Alu.add,
                    )
                nc.sync.dma_start(
                    out=out_dram[:].rearrange("(c p) d -> c p d", p=P)[c], in_=f[:]
                )

            for c in range(NCH):
                feat_chunk(c, sidx_s, ws_s, out_s, "src")

            # ---------------- snap: window gather + argmax ----------------
            xc = sp.tile([P, NCH], f32, tag="xc")
            nc.vector.tensor_scalar(
                out=xc[:], in0=x[:], scalar1=0.0, scalar2=float(H),
                op0=Alu.max, op1=Alu.min,
            )
            yc = sp.tile([P, NCH], f32, tag="yc")
            nc.vector.tensor_scalar(
                out=yc[:], in0=y[:], scalar1=0.0, scalar2=float(W),
                op0=Alu.max, op1=Alu.min,
            )
            xi = _exact_floor(nc, sp, xc, "xi")
            yi = _exact_floor(nc, sp, yc, "yi")

            # window span bases: spanA at (xi, yi), spanB at (xi+2, yi) in padded mp
            wbase = sp.tile([P, NCH, 2], f32, tag="wbase")
            nc.vector.scalar_tensor_tensor(
                out=wbase[:, :, 0], in0=xi[:], scalar=float(S), in1=yi[:],
                op0=Alu.mult, op1=Alu.add,
            )
            nc.vector.tensor_scalar(
                out=wbase[:, :, 1], in0=wbase[:, :, 0], scalar1=float(2 * S),
                scalar2=None, op0=Alu.add,
            )
            wbase_i = sp.tile([P, NCH, 2], i32, tag="wbasei")
            nc.vector.tensor_copy(wbase_i[:], wbase[:])

            mwin = sp.tile([P, NCH, 16], f32, tag="mwin")
            for c in range(NCH):
                spanA = spanp.tile([P, 524], f32, tag="spanA")
                gA = nc.gpsimd.indirect_dma_start(
                    out=spanA[:], out_offset=None, in_=mp[:],
                    in_offset=bass.IndirectOffsetOnAxis(ap=wbase_i[:, c, 0:1], axis=0),
                )
                tile.add_dep_helper(gA.ins, intw.ins, reason="mp written before win gather")
                spanB = spanp.tile([P, 524], f32, tag="spanB")
                gB = nc.gpsimd.indirect_dma_start(
                    out=spanB[:], out_offset=None, in_=mp[:],
                    in_offset=bass.IndirectOffsetOnAxis(ap=wbase_i[:, c, 1:2], axis=0),
                )
                tile.add_dep_helper(gB.ins, intw.ins, reason="mp written before win gather")
                # extract rows: span rows at offsets {0, 520}, 4 cols each
                for half, sp_t in ((0, spanA), (1, spanB)):
                    src = sp_t[:, 0:4]
                    src2 = bass.AP(src.tensor, src.offset, [src.ap[0], [S, 2], [1, 4]])
                    dst = mwin[:, c, half * 8 : half * 8 + 8]
                    dst2 = bass.AP(dst.tensor, dst.offset, [dst.ap[0], [4, 2], [1, 4]])
                    nc.vector.tensor_copy(out=dst2, in_=src2)

            swin = sp.tile([P, NCH, 16], f32, tag="swin")
            nc.vector.tensor_tensor(
                out=swin[:], in0=mwin[:], in1=wv[:].to_broadcast([P, NCH, 16]), op=Alu.mult
            )
            mx = sp.tile([P, NCH, 1], f32, tag="mx")
            nc.vector.tensor_reduce(out=mx[:], in_=swin[:], axis=Ax.X, op=Alu.max)
            eq = sp.tile([P, NCH, 16], f32, tag="eq")
            nc.vector.tensor_tensor(
                out=eq[:], in0=swin[:], in1=mx[:].to_broadcast([P, NCH, 16]), op=Alu.is_ge
            )
            cand = sp.tile([P, NCH, 16], f32, tag="cand")
            nc.vector.tensor_tensor(
                out=cand[:], in0=eq[:], in1=jc[:].to_broadcast([P, NCH, 16]), op=Alu.mult
            )
            nc.vector.tensor_scalar(
                out=cand[:], in0=cand[:], scalar1=-1.0, scalar2=16.0,
                op0=Alu.mult, op1=Alu.add,
            )
            kf = sp.tile([P, NCH], f32, tag="kf")
            nc.vector.tensor_reduce(out=kf[:], in_=cand[:], axis=Ax.X, op=Alu.min)

            # a = k//4 = (k>=4)+(k>=8)+(k>=12); b = k - 4a
            ka = sp.tile([P, NCH], f32, tag="ka")
            nc.vector.tensor_scalar(out=ka[:], in0=kf[:], scalar1=4.0, scalar2=None, op0=Alu.is_ge)
            kt = sp.tile([P, NCH], f32, tag="kt")
            for thr in (8.0, 12.0):
                nc.vector.tensor_scalar(out=kt[:], in0=kf[:], scalar1=thr, scalar2=None, op0=Alu.is_ge)
                nc.vector.tensor_tensor(out=ka[:], in0=ka[:], in1=kt[:], op=Alu.add)
            kb = sp.tile([P, NCH], f32, tag="kb")
            nc.vector.scalar_tensor_tensor(
                out=kb[:], in0=ka[:], scalar=-4.0, in1=kf[:], op0=Alu.mult, op1=Alu.add
            )
            # x_new = xi + a - 2 ; y_new = yi + b - 2
            xn = sp.tile([P, NCH], f32, tag="xn")
            nc.vector.scalar_tensor_tensor(
                out=xn[:], in0=ka[:], scalar=-2.0, in1=xi[:], op0=Alu.add, op1=Alu.add
            )
            yn = sp.tile([P, NCH], f32, tag="yn")
            nc.vector.scalar_tensor_tensor(
                out=yn[:], in0=kb[:], scalar=-2.0, in1=yi[:], op0=Alu.add, op1=Alu.add
            )
            # keep (x==0)&(y==0) -> passthrough
            ex = sp.tile([P, NCH], f32, tag="ex")
            nc.vector.tensor_scalar(out=ex[:], in0=x[:], scalar1=0.0, scalar2=None, op0=Alu.is_equal)
            ey = sp.tile([P, NCH], f32, tag="ey")
            nc.vector.tensor_scalar(out=ey[:], in0=y[:], scalar1=0.0, scalar2=None, op0=Alu.is_equal)
            keep = sp.tile([P, NCH], f32, tag="keep")
            nc.vector.tensor_tensor(out=keep[:], in0=ex[:], in1=ey[:], op=Alu.mult)
            keep_i = sp.tile([P, NCH], i32, tag="keepi")
            nc.vector.tensor_copy(keep_i[:], keep[:])
            ptx = sp.tile([P, NCH], f32, tag="ptx")
            nc.vector.tensor_copy(ptx[:], xn[:])
            nc.vector.copy_predicated(ptx[:], keep_i[:], x[:])
            pty = sp.tile([P, NCH], f32, tag="pty")
            nc.vector.tensor_copy(pty[:], yn[:])
            nc.vector.copy_predicated(pty[:], keep_i[:], y[:])

            po = sp.tile([P, NCH, 2], f32, tag="po")
            nc.vector.tensor_copy(po[:, :, 0], ptx[:])
            nc.vector.tensor_copy(po[:, :, 1], pty[:])
            nc.sync.dma_start(
                out=out_p[:].rearrange("(c p) two -> p c two", p=P), in_=po[:]
            )

            # ---------------- target grid setup + gather/accum ----------------
            sidx_t, ws_t = _bilinear_setup(nc, sp, ptx, pty, "tgt")
            for c in range(NCH):
                feat_chunk(c, sidx_t, ws_t, out_t, "tgt")

    nc.finalize()
    return nc


def _host_consts():
    t = np.arange(16)
    a = t // 4 - 2
    b = t % 4 - 2
    within = ((a * a + b * b) <= 4).astype(np.float32)
    wv = np.ascontiguousarray(np.tile(within[None, None, :], (P, 1, 1)))
    jc = np.ascontiguousarray(
        np.tile((16.0 - t).astype(np.float32)[None, None, :], (P, 1, 1))
    )
    return wv, jc


def _shard_inputs(feature_maps, sample_points, mask_scores):
    wv, jc = _host_consts()
    in_maps = []
    for core in range(8):
        b, half = divmod(core, 2)
        feat_t = np.ascontiguousarray(
            feature_maps[b].transpose(1, 2, 0)
        ).reshape(H * W, D)
        pts = sample_points[b, half * NPTS : (half + 1) * NPTS]
        in_maps.append(
            {
                "feat": feat_t,
                "mask": np.ascontiguousarray(mask_scores[b]),
                "px": np.ascontiguousarray(pts[:, 0]),
                "py": np.ascontiguousarray(pts[:, 1]),
                "withinv": wv,
                "sixteen_minus_j": jc,
            }
        )
    return in_maps


def run(feature_maps, sample_points, mask_scores, trace=False):
    if "nc" not in _CACHE:
        _CACHE["nc"] = build_module()
    nc = _CACHE["nc"]
    in_maps = _shard_inputs(feature_maps, sample_points, mask_scores)
    res = run_bass_kernel_spmd(nc, in_maps, core_ids=list(range(8)), trace=trace)
    B, N = 4, 4096
    tgt = np.empty((B, N, D), np.float32)
    src = np.empty((B, N, D), np.float32)
    pt = np.empty((B, N, 2), np.float32)
    for core in range(8):
        b, half = divmod(core, 2)
        sl = slice(half * NPTS, (half + 1) * NPTS)
        r = res.results[core]
        tgt[b, sl] = r["out_t"]
        src[b, sl] = r["out_s"]
        pt[b, sl] = r["out_p"]
    return (tgt, pt, src), res


def kernel(feature_maps, sample_points, mask_scores):
    outs, _ = run(feature_maps, sample_points, mask_scores)
    return outs


# revision 8
# speedup vs baseline: 1.2129x; 1.2129x over previous
"""Bass/Trainium2 kernel for nn_BilinearSampler (topk_masking).

Contract: kernel(**inputs) takes FULL numpy inputs
  feature_maps [4,256,512,512] f32, sample_points [4,4096,2] f32,
  mask_scores [4,2,512,512] f32
and returns the full reference output tuple:
  (sampled_features_target [4,4096,256], point [4,4096,2],
   sampled_features_source [4,4096,256])

Sharding: 8 cores = 4 batches x 2 halves of the 4096 points.

Feature layout: two row-parity interleaved copies (even-pair blocks and
odd-pair blocks) of the site-major [H,W,D] map, concatenated into one
[2*(H/2)*W, 2*D] table whose rows are full 2x2 bilinear patches
(2 rows x 1 col x D, adjacent cols adjacent rows) — so ONE indirect-DMA
index per point fetches the whole 4 KiB patch. The 4x4 mask argmax
window is fetched as one 4-row span per point from a border-padded
(-1e30) mask-sum plane built on device.
"""

import sys

import numpy as np

try:
    import concourse.bass as bass
except ImportError:  # pragma: no cover
    sys.path.insert(0, "/opt/trn_rl_repo")
    import concourse.bass as bass

import concourse.bacc as bacc
import concourse.mybir as mybir
import concourse.tile as tile
from concourse.bass_utils import run_bass_kernel_spmd

P = 128
NPTS = 2048              # points per core
NCH = NPTS // P          # 16 chunks
H = W = 512
D = 256
S = 520                  # padded mask row stride (and col count)
MP_FLAT = 128 * 2115     # 270720 >= 520*520, 128-partition padded
NEG = -1.0e30
NBLK = H // 2 * W        # blocks per parity copy = 131072
SPAN = 3 * S + 8         # 1568: 4 window rows live at offsets {0,S,2S,3S}+0..3

f32 = mybir.dt.float32
i32 = mybir.dt.int32
Alu = mybir.AluOpType
Ax = mybir.AxisListType

_CACHE = {}


def _exact_floor(nc, pool, v, name):
    """floor(v) for an f32 tile, robust to the cast's rounding mode."""
    shp = list(v.shape)
    ti = pool.tile(shp, i32, tag=f"{name}_i")
    nc.vector.tensor_copy(ti[:], v[:])
    tf = pool.tile(shp, f32, tag=f"{name}_f")
    nc.vector.tensor_copy(tf[:], ti[:])
    gt = pool.tile(shp, f32, tag=f"{name}_gt")
    nc.vector.tensor_tensor(out=gt[:], in0=tf[:], in1=v[:], op=Alu.is_gt)
    fl = pool.tile(shp, f32, tag=f"{name}_fl")
    nc.vector.tensor_tensor(out=fl[:], in0=tf[:], in1=gt[:], op=Alu.subtract)
    return fl


def _grid_to_pix(nc, pool, coord, scale, name):
    """Mirror reference float ops: g = coord/256 - 1; pix = ((g+1)*scale-1)*0.5."""
    g = pool.tile(list(coord.shape), f32, tag=f"{name}_g")
    nc.vector.tensor_scalar(
        out=g[:], in0=coord[:], scalar1=1.0 / 256.0, scalar2=-1.0,
        op0=Alu.mult, op1=Alu.add,
    )
    pix = pool.tile(list(coord.shape), f32, tag=f"{name}_pix")
    nc.vector.tensor_scalar(
        out=pix[:], in0=g[:], scalar1=1.0, scalar2=float(scale),
        op0=Alu.add, op1=Alu.mult,
    )
    nc.vector.tensor_scalar(
        out=pix[:], in0=pix[:], scalar1=-1.0, scalar2=0.5,
        op0=Alu.add, op1=Alu.mult,
    )
    return pix


def _axis_slot_weights(nc, pool, x0f, fr1, xs, name):
    """Slot weights along one axis given clamped fetch base xs.

    slot0 holds the value at coord xs, slot1 at xs+1; corners are x0
    (weight 1-fr1) and x0+1 (weight fr1), zeroed when out of [0, lim-1].
    """
    shp = list(x0f.shape)
    d = pool.tile(shp, f32, tag=f"{name}_d")
    nc.vector.tensor_tensor(out=d[:], in0=x0f[:], in1=xs[:], op=Alu.subtract)
    e0 = pool.tile(shp, f32, tag=f"{name}_e0")
    nc.vector.tensor_scalar(out=e0[:], in0=d[:], scalar1=0.0, scalar2=None, op0=Alu.is_equal)
    em = pool.tile(shp, f32, tag=f"{name}_em")
    nc.vector.tensor_scalar(out=em[:], in0=d[:], scalar1=-1.0, scalar2=None, op0=Alu.is_equal)
    ep = pool.tile(shp, f32, tag=f"{name}_ep")
    nc.vector.tensor_scalar(out=ep[:], in0=d[:], scalar1=1.0, scalar2=None, op0=Alu.is_equal)
    fr0 = pool.tile(shp, f32, tag=f"{name}_fr0")
    nc.vector.tensor_scalar(
        out=fr0[:], in0=fr1[:], scalar1=-1.0, scalar2=1.0, op0=Alu.mult, op1=Alu.add
    )
    a0 = pool.tile(shp, f32, tag=f"{name}_a0")
    nc.vector.tensor_tensor(out=a0[:], in0=fr0[:], in1=e0[:], op=Alu.mult)
    t = pool.tile(shp, f32, tag=f"{name}_t")
    nc.vector.tensor_tensor(out=t[:], in0=fr1[:], in1=em[:], op=Alu.mult)
    nc.vector.tensor_tensor(out=a0[:], in0=a0[:], in1=t[:], op=Alu.add)
    a1 = pool.tile(shp, f32, tag=f"{name}_a1")
    nc.vector.tensor_tensor(out=a1[:], in0=fr1[:], in1=e0[:], op=Alu.mult)
    nc.vector.tensor_tensor(out=t[:], in0=fr0[:], in1=ep[:], op=Alu.mult)
    nc.vector.tensor_tensor(out=a1[:], in0=a1[:], in1=t[:], op=Alu.add)
    return a0, a1


def _bilinear_sites(nc, pool, gx, gy, name):
    """Patch-table row index per point (int32 [P,NCH,1]) + floor/frac state."""
    ix = _grid_to_pix(nc, pool, gx, W, f"{name}_ix")
    iy = _grid_to_pix(nc, pool, gy, H, f"{name}_iy")
    x0 = _exact_floor(nc, pool, ix, f"{name}_x0")
    y0 = _exact_floor(nc, pool, iy, f"{name}_y0")
    xs = pool.tile([P, NCH], f32, tag=f"{name}_xs")
    nc.vector.tensor_scalar(
        out=xs[:], in0=x0[:], scalar1=0.0, scalar2=float(W - 2),
        op0=Alu.max, op1=Alu.min,
    )
    ys = pool.tile([P, NCH], f32, tag=f"{name}_ys")
    nc.vector.tensor_scalar(
        out=ys[:], in0=y0[:], scalar1=0.0, scalar2=float(H - 2),
        op0=Alu.max, op1=Alu.min,
    )
    # parity-pair block: j = ys//2, par = ys - 2j, row = par*NBLK + j*W + xs
    jh = pool.tile([P, NCH], f32, tag=f"{name}_jh")
    nc.vector.tensor_scalar(out=jh[:], in0=ys[:], scalar1=0.5, scalar2=None, op0=Alu.mult)
    j = _exact_floor(nc, pool, jh, f"{name}_j")
    par = pool.tile([P, NCH], f32, tag=f"{name}_par")
    nc.vector.scalar_tensor_tensor(
        out=par[:], in0=j[:], scalar=-2.0, in1=ys[:], op0=Alu.mult, op1=Alu.add
    )
    sidx = pool.tile([P, NCH, 1], f32, tag=f"{name}_sidx")
    nc.vector.scalar_tensor_tensor(
        out=sidx[:, :, 0], in0=j[:], scalar=float(W), in1=xs[:],
        op0=Alu.mult, op1=Alu.add,
    )
    nc.vector.scalar_tensor_tensor(
        out=sidx[:, :, 0], in0=par[:], scalar=float(NBLK), in1=sidx[:, :, 0],
        op0=Alu.mult, op1=Alu.add,
    )
    sidx_i = pool.tile([P, NCH, 1], i32, tag=f"{name}_sidxi")
    nc.vector.tensor_copy(sidx_i[:], sidx[:])
    return sidx_i, ix, iy, x0, y0, xs, ys


def _bilinear_weights(nc, pool, st, name):
    """4 slot weights [P,NCH] in patch-slot order [x0r0, x0r1, x1r0, x1r1]."""
    _, ix, iy, x0, y0, xs, ys = st
    fx1 = pool.tile([P, NCH], f32, tag=f"{name}_fx1")
    nc.vector.tensor_tensor(out=fx1[:], in0=ix[:], in1=x0[:], op=Alu.subtract)
    fy1 = pool.tile([P, NCH], f32, tag=f"{name}_fy1")
    nc.vector.tensor_tensor(out=fy1[:], in0=iy[:], in1=y0[:], op=Alu.subtract)
    a0, a1 = _axis_slot_weights(nc, pool, x0, fx1, xs, f"{name}_x")
    b0, b1 = _axis_slot_weights(nc, pool, y0, fy1, ys, f"{name}_y")
    ws = []
    for wi, (ca, cb) in enumerate(((a0, b0), (a0, b1), (a1, b0), (a1, b1))):
        wt = pool.tile([P, NCH], f32, tag=f"{name}_w{wi}")
        nc.vector.tensor_tensor(out=wt[:], in0=ca[:], in1=cb[:], op=Alu.mult)
        ws.append(wt)
    return ws


def build_module():
    nc = bacc.Bacc("TRN2", target_bir_lowering=False, debug=False, num_devices=8)

    feat2 = nc.dram_tensor("feat2", [2 * NBLK, 2 * D], f32, kind="ExternalInput")
    mask = nc.dram_tensor("mask", [2, H, W], f32, kind="ExternalInput")
    px_in = nc.dram_tensor("px", [P, NCH], f32, kind="ExternalInput")
    py_in = nc.dram_tensor("py", [P, NCH], f32, kind="ExternalInput")
    wv_in = nc.dram_tensor("withinv", [P, 1, 16], f32, kind="ExternalInput")
    jc_in = nc.dram_tensor("sixteen_minus_j", [P, 1, 16], f32, kind="ExternalInput")
    out_t = nc.dram_tensor("out_t", [NPTS, D], f32, kind="ExternalOutput")
    out_s = nc.dram_tensor("out_s", [NPTS, D], f32, kind="ExternalOutput")
    out_p = nc.dram_tensor("out_p", [P, NCH, 2], f32, kind="ExternalOutput")
    mp = nc.dram_tensor("mp", [MP_FLAT, 1], f32)

    mp_h = mp[:].tensor

    with tile.TileContext(nc) as tc:
        with (
            tc.tile_pool(name="consts", bufs=1) as cpool,
            tc.tile_pool(name="scal", bufs=1) as sp,
            tc.tile_pool(name="maskp", bufs=1) as mkp,
            tc.tile_pool(name="span", bufs=4) as spanp,
            tc.tile_pool(name="patch", bufs=6) as patp,
            tc.tile_pool(name="fout", bufs=6) as fp,
        ):
            # ---- points + consts first (unblock DVE + gpsimd quickly) ----
            x = sp.tile([P, NCH], f32, tag="x")
            nc.sync.dma_start(out=x[:], in_=px_in[:])
            y = sp.tile([P, NCH], f32, tag="y")
            nc.sync.dma_start(out=y[:], in_=py_in[:])
            wv = cpool.tile([P, 1, 16], f32)
            nc.sync.dma_start(out=wv[:], in_=wv_in[:])
            jc = cpool.tile([P, 1, 16], f32)
            nc.sync.dma_start(out=jc[:], in_=jc_in[:])

            # ---- source sites -> source gathers start ASAP ----
            st_s = _bilinear_sites(nc, sp, x, y, "src")
            sidx_s = st_s[0]

            def feat_chunk(c, sidx_i, ws, out_dram):
                patch = patp.tile([P, 4 * D], f32, tag="patch")
                nc.gpsimd.indirect_dma_start(
                    out=patch[:], out_offset=None, in_=feat2[:],
                    in_offset=bass.IndirectOffsetOnAxis(ap=sidx_i[:, c, 0:1], axis=0),
                )
                f = fp.tile([P, D], f32, tag="f")
                # exact reference association: ((v00+v01)+v10)+v11 with
                # patch slots [x0r0, x0r1, x1r0, x1r1]
                nc.vector.tensor_scalar(
                    out=f[:], in0=patch[:, 0:D], scalar1=ws[0][:, c : c + 1],
                    scalar2=None, op0=Alu.mult,
                )
                for wi, off in ((2, 2 * D), (1, D), (3, 3 * D)):
                    nc.vector.scalar_tensor_tensor(
                        out=f[:], in0=patch[:, off : off + D],
                        scalar=ws[wi][:, c : c + 1], in1=f[:],
                        op0=Alu.mult, op1=Alu.add,
                    )
                nc.sync.dma_start(
                    out=out_dram[:].rearrange("(c p) d -> c p d", p=P)[c], in_=f[:]
                )

            ws_s = _bilinear_weights(nc, sp, st_s, "src")
            for c in range(NCH):
                feat_chunk(c, sidx_s, ws_s, out_s)

            # ---- padded mask plane (scalar-engine DMAs) ----
            negt = cpool.tile([P, 2115], f32)
            nc.vector.memset(negt[:], NEG)
            mp_flat_ap = bass.AP(mp_h, 0, [[2115, P], [1, 2115]])
            fill = nc.scalar.dma_start(out=mp_flat_ap, in_=negt[:])
            mload = mkp.tile([P, 2, 4, W], f32)
            mask_ap = bass.AP(mask[:].tensor, 0, [[W, P], [H * W, 2], [P * W, 4], [1, W]])
            nc.scalar.dma_start(out=mload[:], in_=mask_ap)
            msum = mkp.tile([P, 4, W], f32)
            nc.vector.tensor_tensor(
                out=msum[:], in0=mload[:, 0], in1=mload[:, 1], op=Alu.add
            )
            mp_int_ap = bass.AP(mp_h, 2 * S + 2, [[S, P], [P * S, 4], [1, W]])
            intw = nc.scalar.dma_start(out=mp_int_ap, in_=msum[:])
            tile.add_dep_helper(intw.ins, fill.ins, reason="mp fill before interior")

            # ---- snap window bases ----
            xc = sp.tile([P, NCH], f32, tag="xc")
            nc.vector.tensor_scalar(
                out=xc[:], in0=x[:], scalar1=0.0, scalar2=float(H),
                op0=Alu.max, op1=Alu.min,
            )
            yc = sp.tile([P, NCH], f32, tag="yc")
            nc.vector.tensor_scalar(
                out=yc[:], in0=y[:], scalar1=0.0, scalar2=float(W),
                op0=Alu.max, op1=Alu.min,
            )
            xi = _exact_floor(nc, sp, xc, "xi")
            yi = _exact_floor(nc, sp, yc, "yi")
            wbase = sp.tile([P, NCH, 1], f32, tag="wbase")
            nc.vector.scalar_tensor_tensor(
                out=wbase[:, :, 0], in0=xi[:], scalar=float(S), in1=yi[:],
                op0=Alu.mult, op1=Alu.add,
            )
            wbase_i = sp.tile([P, NCH, 1], i32, tag="wbasei")
            nc.vector.tensor_copy(wbase_i[:], wbase[:])

            # ---- window gathers: one 4-row span per chunk ----
            mwin = sp.tile([P, NCH, 16], f32, tag="mwin")
            for c in range(NCH):
                span = spanp.tile([P, SPAN], f32, tag="span")
                g = nc.gpsimd.indirect_dma_start(
                    out=span[:], out_offset=None, in_=mp[:],
                    in_offset=bass.IndirectOffsetOnAxis(ap=wbase_i[:, c, 0:1], axis=0),
                )
                tile.add_dep_helper(g.ins, intw.ins, reason="mp before win gather")
                src = span[:, 0:4]
                src2 = bass.AP(src.tensor, src.offset, [src.ap[0], [S, 4], [1, 4]])
                dst = mwin[:, c, :]
                dst2 = bass.AP(dst.tensor, dst.offset, [dst.ap[0], [4, 4], [1, 4]])
                nc.vector.tensor_copy(out=dst2, in_=src2)

            # ---- disk-masked first-index argmax ----
            swin = sp.tile([P, NCH, 16], f32, tag="swin")
            nc.vector.tensor_tensor(
                out=swin[:], in0=mwin[:], in1=wv[:].to_broadcast([P, NCH, 16]), op=Alu.mult
            )
            mx = sp.tile([P, NCH, 1], f32, tag="mx")
            nc.vector.tensor_reduce(out=mx[:], in_=swin[:], axis=Ax.X, op=Alu.max)
            eq = sp.tile([P, NCH, 16], f32, tag="eq")
            nc.vector.tensor_tensor(
                out=eq[:], in0=swin[:], in1=mx[:].to_broadcast([P, NCH, 16]), op=Alu.is_ge
            )
            cand = sp.tile([P, NCH, 16], f32, tag="cand")
            nc.vector.tensor_tensor(
                out=cand[:], in0=eq[:], in1=jc[:].to_broadcast([P, NCH, 16]), op=Alu.mult
            )
            nc.vector.tensor_scalar(
                out=cand[:], in0=cand[:], scalar1=-1.0, scalar2=16.0,
                op0=Alu.mult, op1=Alu.add,
            )
            kf = sp.tile([P, NCH], f32, tag="kf")
            nc.vector.tensor_reduce(out=kf[:], in_=cand[:], axis=Ax.X, op=Alu.min)

            ka = sp.tile([P, NCH], f32, tag="ka")
            nc.vector.tensor_scalar(out=ka[:], in0=kf[:], scalar1=4.0, scalar2=None, op0=Alu.is_ge)
            kt = sp.tile([P, NCH], f32, tag="kt")
            for thr in (8.0, 12.0):
                nc.vector.tensor_scalar(out=kt[:], in0=kf[:], scalar1=thr, scalar2=None, op0=Alu.is_ge)
                nc.vector.tensor_tensor(out=ka[:], in0=ka[:], in1=kt[:], op=Alu.add)
            kb = sp.tile([P, NCH], f32, tag="kb")
            nc.vector.scalar_tensor_tensor(
                out=kb[:], in0=ka[:], scalar=-4.0, in1=kf[:], op0=Alu.mult, op1=Alu.add
            )
            xn = sp.tile([P, NCH], f32, tag="xn")
            nc.vector.scalar_tensor_tensor(
                out=xn[:], in0=ka[:], scalar=-2.0, in1=xi[:], op0=Alu.add, op1=Alu.add
            )
            yn = sp.tile([P, NCH], f32, tag="yn")
            nc.vector.scalar_tensor_tensor(
                out=yn[:], in0=kb[:], scalar=-2.0, in1=yi[:], op0=Alu.add, op1=Alu.add
            )
            ex = sp.tile([P, NCH], f32, tag="ex")
            nc.vector.tensor_scalar(out=ex[:], in0=x[:], scalar1=0.0, scalar2=None, op0=Alu.is_equal)
            ey = sp.tile([P, NCH], f32, tag="ey")
            nc.vector.tensor_scalar(out=ey[:], in0=y[:], scalar1=0.0, scalar2=None, op0=Alu.is_equal)
            keep = sp.tile([P, NCH], f32, tag="keep")
            nc.vector.tensor_tensor(out=keep[:], in0=ex[:], in1=ey[:], op=Alu.mult)
            keep_i = sp.tile([P, NCH], i32, tag="keepi")
            nc.vector.tensor_copy(keep_i[:], keep[:])
            ptx = sp.tile([P, NCH], f32, tag="ptx")
            nc.vector.tensor_copy(ptx[:], xn[:])
            nc.vector.copy_predicated(ptx[:], keep_i[:], x[:])
            pty = sp.tile([P, NCH], f32, tag="pty")
            nc.vector.tensor_copy(pty[:], yn[:])
            nc.vector.copy_predicated(pty[:], keep_i[:], y[:])

            po = sp.tile([P, NCH, 2], f32, tag="po")
            nc.vector.tensor_copy(po[:, :, 0], ptx[:])
            nc.vector.tensor_copy(po[:, :, 1], pty[:])
            nc.sync.dma_start(out=out_p[:], in_=po[:])

            # ---- target ----
            st_t = _bilinear_sites(nc, sp, ptx, pty, "tgt")
            ws_t = _bilinear_weights(nc, sp, st_t, "tgt")
            for c in range(NCH):
                feat_chunk(c, st_t[0], ws_t, out_t)

    nc.finalize()
    return nc


def _host_consts():
    t = np.arange(16)
    a = t // 4 - 2
    b = t % 4 - 2
    within = ((a * a + b * b) <= 4).astype(np.float32)
    wv = np.ascontiguousarray(np.tile(within[None, None, :], (P, 1, 1)))
    jc = np.ascontiguousarray(
        np.tile((16.0 - t).astype(np.float32)[None, None, :], (P, 1, 1))
    )
    return wv, jc


def _pair_table(fmap_b):
    """[D,H,W] -> [2*NBLK, 2*D] patch table (even + odd row pairings)."""
    ft = np.ascontiguousarray(fmap_b.transpose(1, 2, 0))  # [H, W, D]
    tab = np.zeros((2, H // 2, W, 2, D), dtype=np.float32)
    tab[0] = ft.reshape(H // 2, 2, W, D).transpose(0, 2, 1, 3)
    tab[1, : H // 2 - 1] = ft[1 : H - 1].reshape(H // 2 - 1, 2, W, D).transpose(
        0, 2, 1, 3
    )
    return tab.reshape(2 * NBLK, 2 * D)


def _shard_inputs(feature_maps, sample_points, mask_scores):
    wv, jc = _host_consts()
    tables = {}
    masks = {}
    in_maps = []
    for core in range(8):
        b, half = divmod(core, 2)
        if b not in tables:
            tables[b] = _pair_table(feature_maps[b])
            masks[b] = np.ascontiguousarray(mask_scores[b])
        pts = sample_points[b, half * NPTS : (half + 1) * NPTS]
        # device layout [p, c]: point n = c*128 + p
        px = np.ascontiguousarray(pts[:, 0].reshape(NCH, P).T)
        py = np.ascontiguousarray(pts[:, 1].reshape(NCH, P).T)
        in_maps.append(
            {
                "feat2": tables[b],
                "mask": masks[b],
                "px": px,
                "py": py,
                "withinv": wv,
                "sixteen_minus_j": jc,
            }
        )
    return in_maps


def run(feature_maps, sample_points, mask_scores, trace=False):
    if "nc" not in _CACHE:
        _CACHE["nc"] = build_module()
    nc = _CACHE["nc"]
    in_maps = _shard_inputs(feature_maps, sample_points, mask_scores)
    res = run_bass_kernel_spmd(nc, in_maps, core_ids=list(range(8)), trace=trace)
    B, N = 4, 4096
    tgt = np.empty((B, N, D), np.float32)
    src = np.empty((B, N, D), np.float32)
    pt = np.empty((B, N, 2), np.float32)
    for core in range(8):
        b, half = divmod(core, 2)
        sl = slice(half * NPTS, (half + 1) * NPTS)
        r = res.results[core]
        tgt[b, sl] = r["out_t"]
        src[b, sl] = r["out_s"]
        pt[b, sl] = r["out_p"].transpose(1, 0, 2).reshape(NPTS, 2)
    return (tgt, pt, src), res


def kernel(feature_maps, sample_points, mask_scores):
    outs, _ = run(feature_maps, sample_points, mask_scores)
    return outs


# revision 12
# speedup vs baseline: 1.2841x; 1.0587x over previous
"""Bass/Trainium2 kernel for nn_BilinearSampler (topk_masking).

Contract: kernel(**inputs) takes FULL numpy inputs
  feature_maps [4,256,512,512] f32, sample_points [4,4096,2] f32,
  mask_scores [4,2,512,512] f32
and returns the full reference output tuple:
  (sampled_features_target [4,4096,256], point [4,4096,2],
   sampled_features_source [4,4096,256])

Sharding: 8 cores = 4 batches x 2 halves of the 4096 points.

Feature layout: two row-parity interleaved copies (even-pair blocks and
odd-pair blocks) of the site-major [H,W,D] map, concatenated into one
[2*(H/2)*W, 2*D] table whose rows are full 2x2 bilinear patches
(2 rows x 1 col x D, adjacent cols adjacent rows) — so ONE indirect-DMA
index per point fetches the whole 4 KiB patch. The 4x4 mask argmax
window is fetched as one 4-row span per point from a border-padded
(-1e30) mask-sum plane built on device.
"""

import sys

import numpy as np

try:
    import concourse.bass as bass
except ImportError:  # pragma: no cover
    sys.path.insert(0, "/opt/trn_rl_repo")
    import concourse.bass as bass

import concourse.bacc as bacc
import concourse.mybir as mybir
import concourse.tile as tile
from concourse.bass_utils import run_bass_kernel_spmd

P = 128
NPTS = 2048              # points per core
NCH = NPTS // P          # 16 chunks
H = W = 512
D = 256
S = 520                  # padded mask row stride (and col count)
MP_FLAT = 128 * 2115     # 270720 >= 520*520, 128-partition padded
NEG = -1.0e30
NBLK = H // 2 * W        # blocks per parity copy = 131072
SPAN = 3 * S + 8         # 1568: 4 window rows live at offsets {0,S,2S,3S}+0..3

f32 = mybir.dt.float32
i32 = mybir.dt.int32
Alu = mybir.AluOpType
Ax = mybir.AxisListType

_CACHE = {}


def _exact_floor(nc, pool, v, name):
    """floor(v) for an f32 tile, robust to the cast's rounding mode."""
    shp = list(v.shape)
    ti = pool.tile(shp, i32, tag=f"{name}_i")
    nc.vector.tensor_copy(ti[:], v[:])
    tf = pool.tile(shp, f32, tag=f"{name}_f")
    nc.vector.tensor_copy(tf[:], ti[:])
    gt = pool.tile(shp, f32, tag=f"{name}_gt")
    nc.vector.tensor_tensor(out=gt[:], in0=tf[:], in1=v[:], op=Alu.is_gt)
    fl = pool.tile(shp, f32, tag=f"{name}_fl")
    nc.vector.tensor_tensor(out=fl[:], in0=tf[:], in1=gt[:], op=Alu.subtract)
    return fl


def _grid_to_pix(nc, pool, coord, scale, name):
    """Mirror reference float ops: g = coord/256 - 1; pix = ((g+1)*scale-1)*0.5."""
    g = pool.tile(list(coord.shape), f32, tag=f"{name}_g")
    nc.vector.tensor_scalar(
        out=g[:], in0=coord[:], scalar1=1.0 / 256.0, scalar2=-1.0,
        op0=Alu.mult, op1=Alu.add,
    )
    pix = pool.tile(list(coord.shape), f32, tag=f"{name}_pix")
    nc.vector.tensor_scalar(
        out=pix[:], in0=g[:], scalar1=1.0, scalar2=float(scale),
        op0=Alu.add, op1=Alu.mult,
    )
    nc.vector.tensor_scalar(
        out=pix[:], in0=pix[:], scalar1=-1.0, scalar2=0.5,
        op0=Alu.add, op1=Alu.mult,
    )
    return pix


def _axis_slot_weights(nc, pool, x0f, fr1, xs, name):
    """Slot weights along one axis given clamped fetch base xs.

    slot0 holds the value at coord xs, slot1 at xs+1; corners are x0
    (weight 1-fr1) and x0+1 (weight fr1), zeroed when out of [0, lim-1].
    """
    shp = list(x0f.shape)
    d = pool.tile(shp, f32, tag=f"{name}_d")
    nc.vector.tensor_tensor(out=d[:], in0=x0f[:], in1=xs[:], op=Alu.subtract)
    e0 = pool.tile(shp, f32, tag=f"{name}_e0")
    nc.vector.tensor_scalar(out=e0[:], in0=d[:], scalar1=0.0, scalar2=None, op0=Alu.is_equal)
    em = pool.tile(shp, f32, tag=f"{name}_em")
    nc.vector.tensor_scalar(out=em[:], in0=d[:], scalar1=-1.0, scalar2=None, op0=Alu.is_equal)
    ep = pool.tile(shp, f32, tag=f"{name}_ep")
    nc.vector.tensor_scalar(out=ep[:], in0=d[:], scalar1=1.0, scalar2=None, op0=Alu.is_equal)
    fr0 = pool.tile(shp, f32, tag=f"{name}_fr0")
    nc.vector.tensor_scalar(
        out=fr0[:], in0=fr1[:], scalar1=-1.0, scalar2=1.0, op0=Alu.mult, op1=Alu.add
    )
    a0 = pool.tile(shp, f32, tag=f"{name}_a0")
    nc.vector.tensor_tensor(out=a0[:], in0=fr0[:], in1=e0[:], op=Alu.mult)
    t = pool.tile(shp, f32, tag=f"{name}_t")
    nc.vector.tensor_tensor(out=t[:], in0=fr1[:], in1=em[:], op=Alu.mult)
    nc.vector.tensor_tensor(out=a0[:], in0=a0[:], in1=t[:], op=Alu.add)
    a1 = pool.tile(shp, f32, tag=f"{name}_a1")
    nc.vector.tensor_tensor(out=a1[:], in0=fr1[:], in1=e0[:], op=Alu.mult)
    nc.vector.tensor_tensor(out=t[:], in0=fr0[:], in1=ep[:], op=Alu.mult)
    nc.vector.tensor_tensor(out=a1[:], in0=a1[:], in1=t[:], op=Alu.add)
    return a0, a1


def _bilinear_sites(nc, pool, gx, gy, name):
    """Patch-table row index per point (int32 [P,NCH,1]) + floor/frac state."""
    ix = _grid_to_pix(nc, pool, gx, W, f"{name}_ix")
    iy = _grid_to_pix(nc, pool, gy, H, f"{name}_iy")
    x0 = _exact_floor(nc, pool, ix, f"{name}_x0")
    y0 = _exact_floor(nc, pool, iy, f"{name}_y0")
    xs = pool.tile([P, NCH], f32, tag=f"{name}_xs")
    nc.vector.tensor_scalar(
        out=xs[:], in0=x0[:], scalar1=0.0, scalar2=float(W - 2),
        op0=Alu.max, op1=Alu.min,
    )
    ys = pool.tile([P, NCH], f32, tag=f"{name}_ys")
    nc.vector.tensor_scalar(
        out=ys[:], in0=y0[:], scalar1=0.0, scalar2=float(H - 2),
        op0=Alu.max, op1=Alu.min,
    )
    # parity-pair block: j = ys//2, par = ys - 2j, row = par*NBLK + j*W + xs
    jh = pool.tile([P, NCH], f32, tag=f"{name}_jh")
    nc.vector.tensor_scalar(out=jh[:], in0=ys[:], scalar1=0.5, scalar2=None, op0=Alu.mult)
    j = _exact_floor(nc, pool, jh, f"{name}_j")
    par = pool.tile([P, NCH], f32, tag=f"{name}_par")
    nc.vector.scalar_tensor_tensor(
        out=par[:], in0=j[:], scalar=-2.0, in1=ys[:], op0=Alu.mult, op1=Alu.add
    )
    sidx = pool.tile([P, NCH, 1], f32, tag=f"{name}_sidx")
    nc.vector.scalar_tensor_tensor(
        out=sidx[:, :, 0], in0=j[:], scalar=float(W), in1=xs[:],
        op0=Alu.mult, op1=Alu.add,
    )
    nc.vector.scalar_tensor_tensor(
        out=sidx[:, :, 0], in0=par[:], scalar=float(NBLK), in1=sidx[:, :, 0],
        op0=Alu.mult, op1=Alu.add,
    )
    sidx_i = pool.tile([P, NCH, 1], i32, tag=f"{name}_sidxi")
    nc.vector.tensor_copy(sidx_i[:], sidx[:])
    return sidx_i, ix, iy, x0, y0, xs, ys


def _bilinear_weights(nc, pool, st, name):
    """4 slot weights [P,NCH] in patch-slot order [x0r0, x0r1, x1r0, x1r1]."""
    _, ix, iy, x0, y0, xs, ys = st
    fx1 = pool.tile([P, NCH], f32, tag=f"{name}_fx1")
    nc.vector.tensor_tensor(out=fx1[:], in0=ix[:], in1=x0[:], op=Alu.subtract)
    fy1 = pool.tile([P, NCH], f32, tag=f"{name}_fy1")
    nc.vector.tensor_tensor(out=fy1[:], in0=iy[:], in1=y0[:], op=Alu.subtract)
    a0, a1 = _axis_slot_weights(nc, pool, x0, fx1, xs, f"{name}_x")
    b0, b1 = _axis_slot_weights(nc, pool, y0, fy1, ys, f"{name}_y")
    ws = []
    for wi, (ca, cb) in enumerate(((a0, b0), (a0, b1), (a1, b0), (a1, b1))):
        wt = pool.tile([P, NCH], f32, tag=f"{name}_w{wi}")
        nc.vector.tensor_tensor(out=wt[:], in0=ca[:], in1=cb[:], op=Alu.mult)
        ws.append(wt)
    return ws


def build_module():
    nc = bacc.Bacc("TRN2", target_bir_lowering=False, debug=False, num_devices=8)

    feat2 = nc.dram_tensor("feat2", [2 * NBLK, 2 * D], f32, kind="ExternalInput")
    mask = nc.dram_tensor("mask", [2, H, W], f32, kind="ExternalInput")
    px_in = nc.dram_tensor("px", [P, NCH], f32, kind="ExternalInput")
    py_in = nc.dram_tensor("py", [P, NCH], f32, kind="ExternalInput")
    wv_in = nc.dram_tensor("withinv", [P, 1, 16], f32, kind="ExternalInput")
    jc_in = nc.dram_tensor("sixteen_minus_j", [P, 1, 16], f32, kind="ExternalInput")
    out_t = nc.dram_tensor("out_t", [NPTS, D], f32, kind="ExternalOutput")
    out_s = nc.dram_tensor("out_s", [NPTS, D], f32, kind="ExternalOutput")
    out_p = nc.dram_tensor("out_p", [P, NCH, 2], f32, kind="ExternalOutput")
    mp = nc.dram_tensor("mp", [MP_FLAT, 1], f32)

    mp_h = mp[:].tensor

    with tile.TileContext(nc) as tc:
        with (
            tc.tile_pool(name="consts", bufs=1) as cpool,
            tc.tile_pool(name="scal", bufs=1) as sp,
            tc.tile_pool(name="maskp", bufs=1) as mkp,
            tc.tile_pool(name="span", bufs=3) as spanp,
            tc.tile_pool(name="patch", bufs=16) as patp,
            tc.tile_pool(name="fout", bufs=4) as fp,
        ):
            # ---- points + consts first (unblock DVE + gpsimd quickly) ----
            x = sp.tile([P, NCH], f32, tag="x")
            nc.sync.dma_start(out=x[:], in_=px_in[:])
            y = sp.tile([P, NCH], f32, tag="y")
            nc.sync.dma_start(out=y[:], in_=py_in[:])
            wv = cpool.tile([P, 1, 16], f32)
            nc.sync.dma_start(out=wv[:], in_=wv_in[:])
            jc = cpool.tile([P, 1, 16], f32)
            nc.sync.dma_start(out=jc[:], in_=jc_in[:])

            # ---- source sites -> source gathers start ASAP ----
            st_s = _bilinear_sites(nc, sp, x, y, "src")
            sidx_s = st_s[0]

            def gather_chunk(c, sidx_i):
                patch = patp.tile([P, 4 * D], f32, tag="patch")
                nc.gpsimd.indirect_dma_start(
                    out=patch[:], out_offset=None, in_=feat2[:],
                    in_offset=bass.IndirectOffsetOnAxis(ap=sidx_i[:, c, 0:1], axis=0),
                )
                return patch

            def sum_chunks(patches, ws, out_dram, g):
                # 4-chunk group: weighted patch sums then one 512KiB store.
                # exact reference association: ((v00+v01)+v10)+v11 with
                # patch slots [x0r0, x0r1, x1r0, x1r1]
                f4 = fp.tile([P, 4, D], f32, tag="f4")
                for k in range(4):
                    c = 4 * g + k
                    patch = patches[c]
                    nc.scalar.mul(
                        out=f4[:, k, :], in_=patch[:, 0:D], mul=ws[0][:, c : c + 1]
                    )
                    for wi, off in ((2, 2 * D), (1, D), (3, 3 * D)):
                        nc.vector.scalar_tensor_tensor(
                            out=f4[:, k, :], in0=patch[:, off : off + D],
                            scalar=ws[wi][:, c : c + 1], in1=f4[:, k, :],
                            op0=Alu.mult, op1=Alu.add,
                        )
                nc.sync.dma_start(
                    out=out_dram[:].rearrange("(g p) d -> g p d", p=4 * P)[g]
                    .rearrange("(c p) d -> p c d", p=P),
                    in_=f4[:],
                )

            ws_s = _bilinear_weights(nc, sp, st_s, "src")
            src_patches = [gather_chunk(c, sidx_s) for c in range(NCH)]

            # ---- padded mask plane (scalar-engine DMAs) ----
            negt = cpool.tile([P, 2115], f32)
            nc.vector.memset(negt[:], NEG)
            mp_flat_ap = bass.AP(mp_h, 0, [[2115, P], [1, 2115]])
            fill = nc.scalar.dma_start(out=mp_flat_ap, in_=negt[:])
            mload = mkp.tile([P, 2, 4, W], f32)
            mask_ap = bass.AP(mask[:].tensor, 0, [[W, P], [H * W, 2], [P * W, 4], [1, W]])
            nc.scalar.dma_start(out=mload[:], in_=mask_ap)
            msum = mkp.tile([P, 4, W], f32)
            nc.vector.tensor_tensor(
                out=msum[:], in0=mload[:, 0], in1=mload[:, 1], op=Alu.add
            )
            mp_int_ap = bass.AP(mp_h, 2 * S + 2, [[S, P], [P * S, 4], [1, W]])
            intw = nc.scalar.dma_start(out=mp_int_ap, in_=msum[:])
            tile.add_dep_helper(intw.ins, fill.ins, reason="mp fill before interior")

            # ---- snap window bases ----
            xc = sp.tile([P, NCH], f32, tag="xc")
            nc.vector.tensor_scalar(
                out=xc[:], in0=x[:], scalar1=0.0, scalar2=float(H),
                op0=Alu.max, op1=Alu.min,
            )
            yc = sp.tile([P, NCH], f32, tag="yc")
            nc.vector.tensor_scalar(
                out=yc[:], in0=y[:], scalar1=0.0, scalar2=float(W),
                op0=Alu.max, op1=Alu.min,
            )
            xi = _exact_floor(nc, sp, xc, "xi")
            yi = _exact_floor(nc, sp, yc, "yi")
            wbase = sp.tile([P, NCH, 1], f32, tag="wbase")
            nc.vector.scalar_tensor_tensor(
                out=wbase[:, :, 0], in0=xi[:], scalar=float(S), in1=yi[:],
                op0=Alu.mult, op1=Alu.add,
            )
            wbase_i = sp.tile([P, NCH, 1], i32, tag="wbasei")
            nc.vector.tensor_copy(wbase_i[:], wbase[:])

            # ---- window gathers: one 4-row span per chunk ----
            mwin = sp.tile([P, NCH, 16], f32, tag="mwin")
            for c in range(NCH):
                span = spanp.tile([P, SPAN], f32, tag="span")
                g = nc.gpsimd.indirect_dma_start(
                    out=span[:], out_offset=None, in_=mp[:],
                    in_offset=bass.IndirectOffsetOnAxis(ap=wbase_i[:, c, 0:1], axis=0),
                )
                tile.add_dep_helper(g.ins, intw.ins, reason="mp before win gather")
                src = span[:, 0:4]
                src2 = bass.AP(src.tensor, src.offset, [src.ap[0], [S, 4], [1, 4]])
                dst = mwin[:, c, :]
                dst2 = bass.AP(dst.tensor, dst.offset, [dst.ap[0], [4, 4], [1, 4]])
                nc.vector.tensor_copy(out=dst2, in_=src2)

            # ---- disk-masked first-index argmax ----
            swin = sp.tile([P, NCH, 16], f32, tag="swin")
            nc.vector.tensor_tensor(
                out=swin[:], in0=mwin[:], in1=wv[:].to_broadcast([P, NCH, 16]), op=Alu.mult
            )
            mx = sp.tile([P, NCH, 1], f32, tag="mx")
            nc.vector.tensor_reduce(out=mx[:], in_=swin[:], axis=Ax.X, op=Alu.max)
            eq = sp.tile([P, NCH, 16], f32, tag="eq")
            nc.vector.tensor_tensor(
                out=eq[:], in0=swin[:], in1=mx[:].to_broadcast([P, NCH, 16]), op=Alu.is_ge
            )
            cand = sp.tile([P, NCH, 16], f32, tag="cand")
            nc.vector.tensor_tensor(
                out=cand[:], in0=eq[:], in1=jc[:].to_broadcast([P, NCH, 16]), op=Alu.mult
            )
            nc.vector.tensor_scalar(
                out=cand[:], in0=cand[:], scalar1=-1.0, scalar2=16.0,
                op0=Alu.mult, op1=Alu.add,
            )
            kf = sp.tile([P, NCH], f32, tag="kf")
            nc.vector.tensor_reduce(out=kf[:], in_=cand[:], axis=Ax.X, op=Alu.min)

            ka = sp.tile([P, NCH], f32, tag="ka")
            nc.vector.tensor_scalar(out=ka[:], in0=kf[:], scalar1=4.0, scalar2=None, op0=Alu.is_ge)
            kt = sp.tile([P, NCH], f32, tag="kt")
            for thr in (8.0, 12.0):
                nc.vector.tensor_scalar(out=kt[:], in0=kf[:], scalar1=thr, scalar2=None, op0=Alu.is_ge)
                nc.vector.tensor_tensor(out=ka[:], in0=ka[:], in1=kt[:], op=Alu.add)
            kb = sp.tile([P, NCH], f32, tag="kb")
            nc.vector.scalar_tensor_tensor(
                out=kb[:], in0=ka[:], scalar=-4.0, in1=kf[:], op0=Alu.mult, op1=Alu.add
            )
            xn = sp.tile([P, NCH], f32, tag="xn")
            nc.vector.scalar_tensor_tensor(
                out=xn[:], in0=ka[:], scalar=-2.0, in1=xi[:], op0=Alu.add, op1=Alu.add
            )
            yn = sp.tile([P, NCH], f32, tag="yn")
            nc.vector.scalar_tensor_tensor(
                out=yn[:], in0=kb[:], scalar=-2.0, in1=yi[:], op0=Alu.add, op1=Alu.add
            )
            ex = sp.tile([P, NCH], f32, tag="ex")
            nc.vector.tensor_scalar(out=ex[:], in0=x[:], scalar1=0.0, scalar2=None, op0=Alu.is_equal)
            ey = sp.tile([P, NCH], f32, tag="ey")
            nc.vector.tensor_scalar(out=ey[:], in0=y[:], scalar1=0.0, scalar2=None, op0=Alu.is_equal)
            keep = sp.tile([P, NCH], f32, tag="keep")
            nc.vector.tensor_tensor(out=keep[:], in0=ex[:], in1=ey[:], op=Alu.mult)
            keep_i = sp.tile([P, NCH], i32, tag="keepi")
            nc.vector.tensor_copy(keep_i[:], keep[:])
            ptx = sp.tile([P, NCH], f32, tag="ptx")
            nc.vector.tensor_copy(ptx[:], xn[:])
            nc.vector.copy_predicated(ptx[:], keep_i[:], x[:])
            pty = sp.tile([P, NCH], f32, tag="pty")
            nc.vector.tensor_copy(pty[:], yn[:])
            nc.vector.copy_predicated(pty[:], keep_i[:], y[:])

            po = sp.tile([P, NCH, 2], f32, tag="po")
            nc.vector.tensor_copy(po[:, :, 0], ptx[:])
            nc.vector.tensor_copy(po[:, :, 1], pty[:])
            nc.sync.dma_start(out=out_p[:], in_=po[:])

            # ---- target sites/weights, then gathers ----
            st_t = _bilinear_sites(nc, sp, ptx, pty, "tgt")
            ws_t = _bilinear_weights(nc, sp, st_t, "tgt")
            tgt_patches = [gather_chunk(c, st_t[0]) for c in range(NCH)]

            # ---- weighted sums + grouped stores (DVE tail work) ----
            for g in range(NCH // 4):
                sum_chunks(src_patches, ws_s, out_s, g)
            for g in range(NCH // 4):
                sum_chunks(tgt_patches, ws_t, out_t, g)

    nc.finalize()
    return nc


def _host_consts():
    t = np.arange(16)
    a = t // 4 - 2
    b = t % 4 - 2
    within = ((a * a + b * b) <= 4).astype(np.float32)
    wv = np.ascontiguousarray(np.tile(within[None, None, :], (P, 1, 1)))
    jc = np.ascontiguousarray(
        np.tile((16.0 - t).astype(np.float32)[None, None, :], (P, 1, 1))
    )
    return wv, jc


def _pair_table(fmap_b):
    """[D,H,W] -> [2*NBLK, 2*D] patch table (even + odd row pairings)."""
    ft = np.ascontiguousarray(fmap_b.transpose(1, 2, 0))  # [H, W, D]
    tab = np.zeros((2, H // 2, W, 2, D), dtype=np.float32)
    tab[0] = ft.reshape(H // 2, 2, W, D).transpose(0, 2, 1, 3)
    tab[1, : H // 2 - 1] = ft[1 : H - 1].reshape(H // 2 - 1, 2, W, D).transpose(
        0, 2, 1, 3
    )
    return tab.reshape(2 * NBLK, 2 * D)


def _shard_inputs(feature_maps, sample_points, mask_scores):
    wv, jc = _host_consts()
    tables = {}
    masks = {}
    in_maps = []
    for core in range(8):
        b, half = divmod(core, 2)
        if b not in tables:
            tables[b] = _pair_table(feature_maps[b])
            masks[b] = np.ascontiguousarray(mask_scores[b])
        pts = sample_points[b, half * NPTS : (half + 1) * NPTS]
        # device layout [p, c]: point n = c*128 + p
        px = np.ascontiguousarray(pts[:, 0].reshape(NCH, P).T)
        py = np.ascontiguousarray(pts[:, 1].reshape(NCH, P).T)
        in_maps.append(
            {
                "feat2": tables[b],
                "mask": masks[b],
                "px": px,
                "py": py,
                "withinv": wv,
                "sixteen_minus_j": jc,
            }
        )
    return in_maps


def run(feature_maps, sample_points, mask_scores, trace=False):
    if "nc" not in _CACHE:
        _CACHE["nc"] = build_module()
    nc = _CACHE["nc"]
    in_maps = _shard_inputs(feature_maps, sample_points, mask_scores)
    res = run_bass_kernel_spmd(nc, in_maps, core_ids=list(range(8)), trace=trace)
    B, N = 4, 4096
    tgt = np.empty((B, N, D), np.float32)
    src = np.empty((B, N, D), np.float32)
    pt = np.empty((B, N, 2), np.float32)
    for core in range(8):
        b, half = divmod(core, 2)
        sl = slice(half * NPTS, (half + 1) * NPTS)
        r = res.results[core]
        tgt[b, sl] = r["out_t"]
        src[b, sl] = r["out_s"]
        pt[b, sl] = r["out_p"].transpose(1, 0, 2).reshape(NPTS, 2)
    return (tgt, pt, src), res


def kernel(feature_maps, sample_points, mask_scores):
    outs, _ = run(feature_maps, sample_points, mask_scores)
    return outs


# revision 16
# speedup vs baseline: 1.3407x; 1.0441x over previous
"""Bass/Trainium2 kernel for nn_BilinearSampler (topk_masking).

Contract: kernel(**inputs) takes FULL numpy inputs
  feature_maps [4,256,512,512] f32, sample_points [4,4096,2] f32,
  mask_scores [4,2,512,512] f32
and returns the full reference output tuple:
  (sampled_features_target [4,4096,256], point [4,4096,2],
   sampled_features_source [4,4096,256])

Sharding: 8 cores = 4 batches x 2 halves of the 4096 points.

Feature layout: two row-parity interleaved copies (even-pair blocks and
odd-pair blocks) of the site-major [H,W,D] map, concatenated into one
[2*(H/2)*W, 2*D] table whose rows are full 2x2 bilinear patches
(2 rows x 1 col x D, adjacent cols adjacent rows) — so ONE indirect-DMA
index per point fetches the whole 4 KiB patch. The 4x4 mask argmax
window is fetched as one 4-row span per point from a border-padded
(-1e30) mask-sum plane built on device.
"""

import sys

import numpy as np

try:
    import concourse.bass as bass
except ImportError:  # pragma: no cover
    sys.path.insert(0, "/opt/trn_rl_repo")
    import concourse.bass as bass

import concourse.bacc as bacc
import concourse.mybir as mybir
import concourse.tile as tile
from concourse.bass_utils import run_bass_kernel_spmd

P = 128
NPTS = 2048              # points per core
NCH = NPTS // P          # 16 chunks
H = W = 512
D = 256
S = 520                  # padded mask row stride (and col count)
MP_FLAT = 128 * 2115     # 270720 >= 520*520, 128-partition padded
NEG = -1.0e30
NBLK = H // 2 * W        # blocks per parity copy = 131072
SPAN2 = S + 4            # 524: a 2-row window span (rows at offsets {0,S}+0..3)

f32 = mybir.dt.float32
i32 = mybir.dt.int32
Alu = mybir.AluOpType
Ax = mybir.AxisListType

_CACHE = {}


def _exact_floor(nc, pool, v, name):
    """floor(v) for an f32 tile, robust to the cast's rounding mode."""
    shp = list(v.shape)
    ti = pool.tile(shp, i32, tag=f"{name}_i")
    nc.vector.tensor_copy(ti[:], v[:])
    tf = pool.tile(shp, f32, tag=f"{name}_f")
    nc.vector.tensor_copy(tf[:], ti[:])
    gt = pool.tile(shp, f32, tag=f"{name}_gt")
    nc.vector.tensor_tensor(out=gt[:], in0=tf[:], in1=v[:], op=Alu.is_gt)
    fl = pool.tile(shp, f32, tag=f"{name}_fl")
    nc.vector.tensor_tensor(out=fl[:], in0=tf[:], in1=gt[:], op=Alu.subtract)
    return fl


def _grid_to_pix(nc, pool, coord, scale, name):
    """Mirror reference float ops: g = coord/256 - 1; pix = ((g+1)*scale-1)*0.5."""
    g = pool.tile(list(coord.shape), f32, tag=f"{name}_g")
    nc.vector.tensor_scalar(
        out=g[:], in0=coord[:], scalar1=1.0 / 256.0, scalar2=-1.0,
        op0=Alu.mult, op1=Alu.add,
    )
    pix = pool.tile(list(coord.shape), f32, tag=f"{name}_pix")
    nc.vector.tensor_scalar(
        out=pix[:], in0=g[:], scalar1=1.0, scalar2=float(scale),
        op0=Alu.add, op1=Alu.mult,
    )
    nc.vector.tensor_scalar(
        out=pix[:], in0=pix[:], scalar1=-1.0, scalar2=0.5,
        op0=Alu.add, op1=Alu.mult,
    )
    return pix


def _axis_slot_weights(nc, pool, x0f, fr1, xs, name):
    """Slot weights along one axis given clamped fetch base xs.

    slot0 holds the value at coord xs, slot1 at xs+1; corners are x0
    (weight 1-fr1) and x0+1 (weight fr1), zeroed when out of [0, lim-1].
    """
    shp = list(x0f.shape)
    d = pool.tile(shp, f32, tag=f"{name}_d")
    nc.vector.tensor_tensor(out=d[:], in0=x0f[:], in1=xs[:], op=Alu.subtract)
    e0 = pool.tile(shp, f32, tag=f"{name}_e0")
    nc.vector.tensor_scalar(out=e0[:], in0=d[:], scalar1=0.0, scalar2=None, op0=Alu.is_equal)
    em = pool.tile(shp, f32, tag=f"{name}_em")
    nc.vector.tensor_scalar(out=em[:], in0=d[:], scalar1=-1.0, scalar2=None, op0=Alu.is_equal)
    ep = pool.tile(shp, f32, tag=f"{name}_ep")
    nc.vector.tensor_scalar(out=ep[:], in0=d[:], scalar1=1.0, scalar2=None, op0=Alu.is_equal)
    fr0 = pool.tile(shp, f32, tag=f"{name}_fr0")
    nc.vector.tensor_scalar(
        out=fr0[:], in0=fr1[:], scalar1=-1.0, scalar2=1.0, op0=Alu.mult, op1=Alu.add
    )
    a0 = pool.tile(shp, f32, tag=f"{name}_a0")
    nc.vector.tensor_tensor(out=a0[:], in0=fr0[:], in1=e0[:], op=Alu.mult)
    t = pool.tile(shp, f32, tag=f"{name}_t")
    nc.vector.tensor_tensor(out=t[:], in0=fr1[:], in1=em[:], op=Alu.mult)
    nc.vector.tensor_tensor(out=a0[:], in0=a0[:], in1=t[:], op=Alu.add)
    a1 = pool.tile(shp, f32, tag=f"{name}_a1")
    nc.vector.tensor_tensor(out=a1[:], in0=fr1[:], in1=e0[:], op=Alu.mult)
    nc.vector.tensor_tensor(out=t[:], in0=fr0[:], in1=ep[:], op=Alu.mult)
    nc.vector.tensor_tensor(out=a1[:], in0=a1[:], in1=t[:], op=Alu.add)
    return a0, a1


def _bilinear_sites(nc, pool, gx, gy, name):
    """Patch-table row index per point (int32 [P,NCH,1]) + floor/frac state."""
    ix = _grid_to_pix(nc, pool, gx, W, f"{name}_ix")
    iy = _grid_to_pix(nc, pool, gy, H, f"{name}_iy")
    x0 = _exact_floor(nc, pool, ix, f"{name}_x0")
    y0 = _exact_floor(nc, pool, iy, f"{name}_y0")
    xs = pool.tile([P, NCH], f32, tag=f"{name}_xs")
    nc.vector.tensor_scalar(
        out=xs[:], in0=x0[:], scalar1=0.0, scalar2=float(W - 2),
        op0=Alu.max, op1=Alu.min,
    )
    ys = pool.tile([P, NCH], f32, tag=f"{name}_ys")
    nc.vector.tensor_scalar(
        out=ys[:], in0=y0[:], scalar1=0.0, scalar2=float(H - 2),
        op0=Alu.max, op1=Alu.min,
    )
    # parity-pair block: j = ys//2, par = ys - 2j, row = par*NBLK + j*W + xs
    jh = pool.tile([P, NCH], f32, tag=f"{name}_jh")
    nc.vector.tensor_scalar(out=jh[:], in0=ys[:], scalar1=0.5, scalar2=None, op0=Alu.mult)
    j = _exact_floor(nc, pool, jh, f"{name}_j")
    par = pool.tile([P, NCH], f32, tag=f"{name}_par")
    nc.vector.scalar_tensor_tensor(
        out=par[:], in0=j[:], scalar=-2.0, in1=ys[:], op0=Alu.mult, op1=Alu.add
    )
    sidx = pool.tile([P, NCH, 1], f32, tag=f"{name}_sidx")
    nc.vector.scalar_tensor_tensor(
        out=sidx[:, :, 0], in0=j[:], scalar=float(W), in1=xs[:],
        op0=Alu.mult, op1=Alu.add,
    )
    nc.vector.scalar_tensor_tensor(
        out=sidx[:, :, 0], in0=par[:], scalar=float(NBLK), in1=sidx[:, :, 0],
        op0=Alu.mult, op1=Alu.add,
    )
    sidx_i = pool.tile([P, NCH, 1], i32, tag=f"{name}_sidxi")
    nc.vector.tensor_copy(sidx_i[:], sidx[:])
    return sidx_i, ix, iy, x0, y0, xs, ys


def _bilinear_weights(nc, pool, st, name):
    """4 slot weights [P,NCH] in patch-slot order [x0r0, x0r1, x1r0, x1r1]."""
    _, ix, iy, x0, y0, xs, ys = st
    fx1 = pool.tile([P, NCH], f32, tag=f"{name}_fx1")
    nc.vector.tensor_tensor(out=fx1[:], in0=ix[:], in1=x0[:], op=Alu.subtract)
    fy1 = pool.tile([P, NCH], f32, tag=f"{name}_fy1")
    nc.vector.tensor_tensor(out=fy1[:], in0=iy[:], in1=y0[:], op=Alu.subtract)
    a0, a1 = _axis_slot_weights(nc, pool, x0, fx1, xs, f"{name}_x")
    b0, b1 = _axis_slot_weights(nc, pool, y0, fy1, ys, f"{name}_y")
    ws = []
    for wi, (ca, cb) in enumerate(((a0, b0), (a0, b1), (a1, b0), (a1, b1))):
        wt = pool.tile([P, NCH], f32, tag=f"{name}_w{wi}")
        nc.vector.tensor_tensor(out=wt[:], in0=ca[:], in1=cb[:], op=Alu.mult)
        ws.append(wt)
    return ws


def build_module():
    nc = bacc.Bacc("TRN2", target_bir_lowering=False, debug=False, num_devices=8)

    feat2 = nc.dram_tensor("feat2", [2 * NBLK, 2 * D], f32, kind="ExternalInput")
    mask = nc.dram_tensor("mask", [2, H, W], f32, kind="ExternalInput")
    px_in = nc.dram_tensor("px", [P, NCH], f32, kind="ExternalInput")
    py_in = nc.dram_tensor("py", [P, NCH], f32, kind="ExternalInput")
    wv_in = nc.dram_tensor("withinv", [P, 1, 16], f32, kind="ExternalInput")
    jc_in = nc.dram_tensor("sixteen_minus_j", [P, 1, 16], f32, kind="ExternalInput")
    out_t = nc.dram_tensor("out_t", [NPTS, D], f32, kind="ExternalOutput")
    out_s = nc.dram_tensor("out_s", [NPTS, D], f32, kind="ExternalOutput")
    out_p = nc.dram_tensor("out_p", [P, NCH, 2], f32, kind="ExternalOutput")
    mp = nc.dram_tensor("mp", [MP_FLAT, 1], f32)

    mp_h = mp[:].tensor

    with tile.TileContext(nc) as tc:
        with (
            tc.tile_pool(name="consts", bufs=1) as cpool,
            tc.tile_pool(name="scal", bufs=1) as sp,
            tc.tile_pool(name="maskp", bufs=1) as mkp,
            tc.tile_pool(name="span", bufs=8) as spanp,
            tc.tile_pool(name="patch", bufs=16) as patp,
            tc.tile_pool(name="fout", bufs=8) as fp,
        ):
            # ---- points + consts first (unblock DVE + gpsimd quickly) ----
            x = sp.tile([P, NCH], f32, tag="x")
            nc.sync.dma_start(out=x[:], in_=px_in[:])
            y = sp.tile([P, NCH], f32, tag="y")
            nc.sync.dma_start(out=y[:], in_=py_in[:])
            wv = cpool.tile([P, 1, 16], f32)
            nc.sync.dma_start(out=wv[:], in_=wv_in[:])
            jc = cpool.tile([P, 1, 16], f32)
            nc.sync.dma_start(out=jc[:], in_=jc_in[:])

            # ---- source sites -> source gathers start ASAP ----
            st_s = _bilinear_sites(nc, sp, x, y, "src")
            sidx_s = st_s[0]

            def gather_chunk(c, sidx_i):
                patch = patp.tile([P, 4 * D], f32, tag="patch")
                nc.gpsimd.indirect_dma_start(
                    out=patch[:], out_offset=None, in_=feat2[:],
                    in_offset=bass.IndirectOffsetOnAxis(ap=sidx_i[:, c, 0:1], axis=0),
                )
                return patch

            def sum_chunks(patches, ws, out_dram, g):
                # 4-chunk group: weighted patch sums then one 512KiB store.
                # exact reference association: ((v00+v01)+v10)+v11 with
                # patch slots [x0r0, x0r1, x1r0, x1r1]
                f4 = fp.tile([P, 4, D], f32, tag="f4")
                for k in range(4):
                    c = 4 * g + k
                    patch = patches[c]
                    nc.scalar.mul(
                        out=f4[:, k, :], in_=patch[:, 0:D], mul=ws[0][:, c : c + 1]
                    )
                    for wi, off in ((2, 2 * D), (1, D), (3, 3 * D)):
                        nc.vector.scalar_tensor_tensor(
                            out=f4[:, k, :], in0=patch[:, off : off + D],
                            scalar=ws[wi][:, c : c + 1], in1=f4[:, k, :],
                            op0=Alu.mult, op1=Alu.add,
                        )
                nc.sync.dma_start(
                    out=out_dram[:].rearrange("(g p) d -> g p d", p=4 * P)[g]
                    .rearrange("(c p) d -> p c d", p=P),
                    in_=f4[:],
                )

            ws_s = _bilinear_weights(nc, sp, st_s, "src")
            src_patches = [gather_chunk(c, sidx_s) for c in range(NCH)]

            # ---- padded mask plane (scalar-engine DMAs) ----
            negt = cpool.tile([P, 2115], f32)
            nc.vector.memset(negt[:], NEG)
            mp_flat_ap = bass.AP(mp_h, 0, [[2115, P], [1, 2115]])
            fill = nc.scalar.dma_start(out=mp_flat_ap, in_=negt[:])
            mload = mkp.tile([P, 2, 4, W], f32)
            mask_ap = bass.AP(mask[:].tensor, 0, [[W, P], [H * W, 2], [P * W, 4], [1, W]])
            nc.scalar.dma_start(out=mload[:], in_=mask_ap)
            msum = mkp.tile([P, 4, W], f32)
            nc.vector.tensor_tensor(
                out=msum[:], in0=mload[:, 0], in1=mload[:, 1], op=Alu.add
            )
            mp_int_ap = bass.AP(mp_h, 2 * S + 2, [[S, P], [P * S, 4], [1, W]])
            intw = nc.scalar.dma_start(out=mp_int_ap, in_=msum[:])
            tile.add_dep_helper(intw.ins, fill.ins, reason="mp fill before interior")

            # ---- snap window bases ----
            xc = sp.tile([P, NCH], f32, tag="xc")
            nc.vector.tensor_scalar(
                out=xc[:], in0=x[:], scalar1=0.0, scalar2=float(H),
                op0=Alu.max, op1=Alu.min,
            )
            yc = sp.tile([P, NCH], f32, tag="yc")
            nc.vector.tensor_scalar(
                out=yc[:], in0=y[:], scalar1=0.0, scalar2=float(W),
                op0=Alu.max, op1=Alu.min,
            )
            xi = _exact_floor(nc, sp, xc, "xi")
            yi = _exact_floor(nc, sp, yc, "yi")
            wbase = sp.tile([P, NCH, 2], f32, tag="wbase")
            nc.vector.scalar_tensor_tensor(
                out=wbase[:, :, 0], in0=xi[:], scalar=float(S), in1=yi[:],
                op0=Alu.mult, op1=Alu.add,
            )
            nc.vector.tensor_scalar(
                out=wbase[:, :, 1], in0=wbase[:, :, 0], scalar1=float(2 * S),
                scalar2=None, op0=Alu.add,
            )
            wbase_i = sp.tile([P, NCH, 2], i32, tag="wbasei")
            nc.vector.tensor_copy(wbase_i[:], wbase[:])

            # ---- source sums first on DVE (their data lands earliest) ----
            for g in range(NCH // 4):
                sum_chunks(src_patches, ws_s, out_s, g)

            # ---- window gathers: two 2-row spans per chunk ----
            mwin = sp.tile([P, NCH, 16], f32, tag="mwin")
            for c in range(NCH):
                for half in range(2):
                    span = spanp.tile([P, SPAN2], f32, tag="span")
                    g = nc.gpsimd.indirect_dma_start(
                        out=span[:], out_offset=None, in_=mp[:],
                        in_offset=bass.IndirectOffsetOnAxis(
                            ap=wbase_i[:, c, half : half + 1], axis=0
                        ),
                    )
                    tile.add_dep_helper(g.ins, intw.ins, reason="mp before win gather")
                    src = span[:, 0:4]
                    src2 = bass.AP(src.tensor, src.offset, [src.ap[0], [S, 2], [1, 4]])
                    dst = mwin[:, c, half * 8 : half * 8 + 8]
                    dst2 = bass.AP(dst.tensor, dst.offset, [dst.ap[0], [4, 2], [1, 4]])
                    nc.vector.tensor_copy(out=dst2, in_=src2)

            # ---- disk-masked first-index argmax ----
            swin = sp.tile([P, NCH, 16], f32, tag="swin")
            nc.vector.tensor_tensor(
                out=swin[:], in0=mwin[:], in1=wv[:].to_broadcast([P, NCH, 16]), op=Alu.mult
            )
            mx = sp.tile([P, NCH, 1], f32, tag="mx")
            nc.vector.tensor_reduce(out=mx[:], in_=swin[:], axis=Ax.X, op=Alu.max)
            eq = sp.tile([P, NCH, 16], f32, tag="eq")
            nc.vector.tensor_tensor(
                out=eq[:], in0=swin[:], in1=mx[:].to_broadcast([P, NCH, 16]), op=Alu.is_ge
            )
            cand = sp.tile([P, NCH, 16], f32, tag="cand")
            nc.vector.tensor_tensor(
                out=cand[:], in0=eq[:], in1=jc[:].to_broadcast([P, NCH, 16]), op=Alu.mult
            )
            nc.vector.tensor_scalar(
                out=cand[:], in0=cand[:], scalar1=-1.0, scalar2=16.0,
                op0=Alu.mult, op1=Alu.add,
            )
            kf = sp.tile([P, NCH], f32, tag="kf")
            nc.vector.tensor_reduce(out=kf[:], in_=cand[:], axis=Ax.X, op=Alu.min)

            ka = sp.tile([P, NCH], f32, tag="ka")
            nc.vector.tensor_scalar(out=ka[:], in0=kf[:], scalar1=4.0, scalar2=None, op0=Alu.is_ge)
            kt = sp.tile([P, NCH], f32, tag="kt")
            for thr in (8.0, 12.0):
                nc.vector.tensor_scalar(out=kt[:], in0=kf[:], scalar1=thr, scalar2=None, op0=Alu.is_ge)
                nc.vector.tensor_tensor(out=ka[:], in0=ka[:], in1=kt[:], op=Alu.add)
            kb = sp.tile([P, NCH], f32, tag="kb")
            nc.vector.scalar_tensor_tensor(
                out=kb[:], in0=ka[:], scalar=-4.0, in1=kf[:], op0=Alu.mult, op1=Alu.add
            )
            xn = sp.tile([P, NCH], f32, tag="xn")
            nc.vector.scalar_tensor_tensor(
                out=xn[:], in0=ka[:], scalar=-2.0, in1=xi[:], op0=Alu.add, op1=Alu.add
            )
            yn = sp.tile([P, NCH], f32, tag="yn")
            nc.vector.scalar_tensor_tensor(
                out=yn[:], in0=kb[:], scalar=-2.0, in1=yi[:], op0=Alu.add, op1=Alu.add
            )
            ex = sp.tile([P, NCH], f32, tag="ex")
            nc.vector.tensor_scalar(out=ex[:], in0=x[:], scalar1=0.0, scalar2=None, op0=Alu.is_equal)
            ey = sp.tile([P, NCH], f32, tag="ey")
            nc.vector.tensor_scalar(out=ey[:], in0=y[:], scalar1=0.0, scalar2=None, op0=Alu.is_equal)
            keep = sp.tile([P, NCH], f32, tag="keep")
            nc.vector.tensor_tensor(out=keep[:], in0=ex[:], in1=ey[:], op=Alu.mult)
            keep_i = sp.tile([P, NCH], i32, tag="keepi")
            nc.vector.tensor_copy(keep_i[:], keep[:])
            ptx = sp.tile([P, NCH], f32, tag="ptx")
            nc.vector.tensor_copy(ptx[:], xn[:])
            nc.vector.copy_predicated(ptx[:], keep_i[:], x[:])
            pty = sp.tile([P, NCH], f32, tag="pty")
            nc.vector.tensor_copy(pty[:], yn[:])
            nc.vector.copy_predicated(pty[:], keep_i[:], y[:])

            po = sp.tile([P, NCH, 2], f32, tag="po")
            nc.vector.tensor_copy(po[:, :, 0], ptx[:])
            nc.vector.tensor_copy(po[:, :, 1], pty[:])
            nc.sync.dma_start(out=out_p[:], in_=po[:])

            # ---- target sites/weights, then gathers ----
            st_t = _bilinear_sites(nc, sp, ptx, pty, "tgt")
            ws_t = _bilinear_weights(nc, sp, st_t, "tgt")
            tgt_patches = [gather_chunk(c, st_t[0]) for c in range(NCH)]

            # ---- target sums + grouped stores ----
            for g in range(NCH // 4):
                sum_chunks(tgt_patches, ws_t, out_t, g)

    nc.finalize()
    return nc


def _host_consts():
    t = np.arange(16)
    a = t // 4 - 2
    b = t % 4 - 2
    within = ((a * a + b * b) <= 4).astype(np.float32)
    wv = np.ascontiguousarray(np.tile(within[None, None, :], (P, 1, 1)))
    jc = np.ascontiguousarray(
        np.tile((16.0 - t).astype(np.float32)[None, None, :], (P, 1, 1))
    )
    return wv, jc


def _pair_table(fmap_b):
    """[D,H,W] -> [2*NBLK, 2*D] patch table (even + odd row pairings)."""
    ft = np.ascontiguousarray(fmap_b.transpose(1, 2, 0))  # [H, W, D]
    tab = np.zeros((2, H // 2, W, 2, D), dtype=np.float32)
    tab[0] = ft.reshape(H // 2, 2, W, D).transpose(0, 2, 1, 3)
    tab[1, : H // 2 - 1] = ft[1 : H - 1].reshape(H // 2 - 1, 2, W, D).transpose(
        0, 2, 1, 3
    )
    return tab.reshape(2 * NBLK, 2 * D)


def _shard_inputs(feature_maps, sample_points, mask_scores):
    wv, jc = _host_consts()
    tables = {}
    masks = {}
    in_maps = []
    for core in range(8):
        b, half = divmod(core, 2)
        if b not in tables:
            tables[b] = _pair_table(feature_maps[b])
            masks[b] = np.ascontiguousarray(mask_scores[b])
        pts = sample_points[b, half * NPTS : (half + 1) * NPTS]
        # device layout [p, c]: point n = c*128 + p
        px = np.ascontiguousarray(pts[:, 0].reshape(NCH, P).T)
        py = np.ascontiguousarray(pts[:, 1].reshape(NCH, P).T)
        in_maps.append(
            {
                "feat2": tables[b],
                "mask": masks[b],
                "px": px,
                "py": py,
                "withinv": wv,
                "sixteen_minus_j": jc,
            }
        )
    return in_maps


def run(feature_maps, sample_points, mask_scores, trace=False):
    if "nc" not in _CACHE:
        _CACHE["nc"] = build_module()
    nc = _CACHE["nc"]
    in_maps = _shard_inputs(feature_maps, sample_points, mask_scores)
    res = run_bass_kernel_spmd(nc, in_maps, core_ids=list(range(8)), trace=trace)
    B, N = 4, 4096
    tgt = np.empty((B, N, D), np.float32)
    src = np.empty((B, N, D), np.float32)
    pt = np.empty((B, N, 2), np.float32)
    for core in range(8):
        b, half = divmod(core, 2)
        sl = slice(half * NPTS, (half + 1) * NPTS)
        r = res.results[core]
        tgt[b, sl] = r["out_t"]
        src[b, sl] = r["out_s"]
        pt[b, sl] = r["out_p"].transpose(1, 0, 2).reshape(NPTS, 2)
    return (tgt, pt, src), res


def kernel(feature_maps, sample_points, mask_scores):
    outs, _ = run(feature_maps, sample_points, mask_scores)
    return outs


# revision 19
# speedup vs baseline: 1.4153x; 1.0556x over previous
"""Bass/Trainium2 kernel for nn_BilinearSampler (topk_masking).

Contract: kernel(**inputs) takes FULL numpy inputs
  feature_maps [4,256,512,512] f32, sample_points [4,4096,2] f32,
  mask_scores [4,2,512,512] f32
and returns the full reference output tuple:
  (sampled_features_target [4,4096,256], point [4,4096,2],
   sampled_features_source [4,4096,256])

Sharding: 8 cores = 4 batches x 2 halves of the 4096 points.

Feature layout: two row-parity interleaved copies (even-pair blocks and
odd-pair blocks) of the site-major [H,W,D] map, concatenated into one
[2*(H/2)*W, 2*D] table whose rows are full 2x2 bilinear patches
(2 rows x 1 col x D, adjacent cols adjacent rows) — so ONE indirect-DMA
index per point fetches the whole 4 KiB patch. The 4x4 mask argmax
window is fetched as one 4-row span per point from a border-padded
(-1e30) mask-sum plane built on device.
"""

import sys

import numpy as np

try:
    import concourse.bass as bass
except ImportError:  # pragma: no cover
    sys.path.insert(0, "/opt/trn_rl_repo")
    import concourse.bass as bass

import concourse.bacc as bacc
import concourse.mybir as mybir
import concourse.tile as tile
from concourse.bass_utils import run_bass_kernel_spmd

P = 128
NPTS = 2048              # points per core
NCH = NPTS // P          # 16 chunks
H = W = 512
D = 256
S = 520                  # padded mask row stride (and col count)
MP_FLAT = 128 * 2115     # 270720 >= 520*520, 128-partition padded
NEG = -1.0e30
NBLK = H // 2 * W        # blocks per parity copy = 131072
SPAN4 = 3 * S + 8        # 1568: 4 window rows live at offsets {0,S,2S,3S}+0..3

f32 = mybir.dt.float32
i32 = mybir.dt.int32
Alu = mybir.AluOpType
Ax = mybir.AxisListType

_CACHE = {}


def _exact_floor(nc, pool, v, name):
    """floor(v) for an f32 tile, robust to the cast's rounding mode."""
    shp = list(v.shape)
    ti = pool.tile(shp, i32, tag=f"{name}_i")
    nc.vector.tensor_copy(ti[:], v[:])
    tf = pool.tile(shp, f32, tag=f"{name}_f")
    nc.vector.tensor_copy(tf[:], ti[:])
    gt = pool.tile(shp, f32, tag=f"{name}_gt")
    nc.vector.tensor_tensor(out=gt[:], in0=tf[:], in1=v[:], op=Alu.is_gt)
    fl = pool.tile(shp, f32, tag=f"{name}_fl")
    nc.vector.tensor_tensor(out=fl[:], in0=tf[:], in1=gt[:], op=Alu.subtract)
    return fl


def _grid_to_pix(nc, pool, coord, scale, name):
    """Mirror reference float ops: g = coord/256 - 1; pix = ((g+1)*scale-1)*0.5."""
    g = pool.tile(list(coord.shape), f32, tag=f"{name}_g")
    nc.vector.tensor_scalar(
        out=g[:], in0=coord[:], scalar1=1.0 / 256.0, scalar2=-1.0,
        op0=Alu.mult, op1=Alu.add,
    )
    pix = pool.tile(list(coord.shape), f32, tag=f"{name}_pix")
    nc.vector.tensor_scalar(
        out=pix[:], in0=g[:], scalar1=1.0, scalar2=float(scale),
        op0=Alu.add, op1=Alu.mult,
    )
    nc.vector.tensor_scalar(
        out=pix[:], in0=pix[:], scalar1=-1.0, scalar2=0.5,
        op0=Alu.add, op1=Alu.mult,
    )
    return pix


def _axis_slot_weights(nc, pool, x0f, fr1, xs, name):
    """Slot weights along one axis given clamped fetch base xs.

    slot0 holds the value at coord xs, slot1 at xs+1; corners are x0
    (weight 1-fr1) and x0+1 (weight fr1), zeroed when out of [0, lim-1].
    """
    shp = list(x0f.shape)
    d = pool.tile(shp, f32, tag=f"{name}_d")
    nc.vector.tensor_tensor(out=d[:], in0=x0f[:], in1=xs[:], op=Alu.subtract)
    e0 = pool.tile(shp, f32, tag=f"{name}_e0")
    nc.vector.tensor_scalar(out=e0[:], in0=d[:], scalar1=0.0, scalar2=None, op0=Alu.is_equal)
    em = pool.tile(shp, f32, tag=f"{name}_em")
    nc.vector.tensor_scalar(out=em[:], in0=d[:], scalar1=-1.0, scalar2=None, op0=Alu.is_equal)
    ep = pool.tile(shp, f32, tag=f"{name}_ep")
    nc.vector.tensor_scalar(out=ep[:], in0=d[:], scalar1=1.0, scalar2=None, op0=Alu.is_equal)
    fr0 = pool.tile(shp, f32, tag=f"{name}_fr0")
    nc.vector.tensor_scalar(
        out=fr0[:], in0=fr1[:], scalar1=-1.0, scalar2=1.0, op0=Alu.mult, op1=Alu.add
    )
    a0 = pool.tile(shp, f32, tag=f"{name}_a0")
    nc.vector.tensor_tensor(out=a0[:], in0=fr0[:], in1=e0[:], op=Alu.mult)
    t = pool.tile(shp, f32, tag=f"{name}_t")
    nc.vector.tensor_tensor(out=t[:], in0=fr1[:], in1=em[:], op=Alu.mult)
    nc.vector.tensor_tensor(out=a0[:], in0=a0[:], in1=t[:], op=Alu.add)
    a1 = pool.tile(shp, f32, tag=f"{name}_a1")
    nc.vector.tensor_tensor(out=a1[:], in0=fr1[:], in1=e0[:], op=Alu.mult)
    nc.vector.tensor_tensor(out=t[:], in0=fr0[:], in1=ep[:], op=Alu.mult)
    nc.vector.tensor_tensor(out=a1[:], in0=a1[:], in1=t[:], op=Alu.add)
    return a0, a1


def _bilinear_sites(nc, pool, gx, gy, name):
    """Patch-table row index per point (int32 [P,NCH,1]) + floor/frac state."""
    ix = _grid_to_pix(nc, pool, gx, W, f"{name}_ix")
    iy = _grid_to_pix(nc, pool, gy, H, f"{name}_iy")
    x0 = _exact_floor(nc, pool, ix, f"{name}_x0")
    y0 = _exact_floor(nc, pool, iy, f"{name}_y0")
    xs = pool.tile([P, NCH], f32, tag=f"{name}_xs")
    nc.vector.tensor_scalar(
        out=xs[:], in0=x0[:], scalar1=0.0, scalar2=float(W - 2),
        op0=Alu.max, op1=Alu.min,
    )
    ys = pool.tile([P, NCH], f32, tag=f"{name}_ys")
    nc.vector.tensor_scalar(
        out=ys[:], in0=y0[:], scalar1=0.0, scalar2=float(H - 2),
        op0=Alu.max, op1=Alu.min,
    )
    # parity-pair block: j = ys//2, par = ys - 2j, row = par*NBLK + j*W + xs
    jh = pool.tile([P, NCH], f32, tag=f"{name}_jh")
    nc.vector.tensor_scalar(out=jh[:], in0=ys[:], scalar1=0.5, scalar2=None, op0=Alu.mult)
    j = _exact_floor(nc, pool, jh, f"{name}_j")
    par = pool.tile([P, NCH], f32, tag=f"{name}_par")
    nc.vector.scalar_tensor_tensor(
        out=par[:], in0=j[:], scalar=-2.0, in1=ys[:], op0=Alu.mult, op1=Alu.add
    )
    sidx = pool.tile([P, NCH, 1], f32, tag=f"{name}_sidx")
    nc.vector.scalar_tensor_tensor(
        out=sidx[:, :, 0], in0=j[:], scalar=float(W), in1=xs[:],
        op0=Alu.mult, op1=Alu.add,
    )
    nc.vector.scalar_tensor_tensor(
        out=sidx[:, :, 0], in0=par[:], scalar=float(NBLK), in1=sidx[:, :, 0],
        op0=Alu.mult, op1=Alu.add,
    )
    sidx_i = pool.tile([P, NCH, 1], i32, tag=f"{name}_sidxi")
    nc.vector.tensor_copy(sidx_i[:], sidx[:])
    return sidx_i, ix, iy, x0, y0, xs, ys


def _bilinear_weights(nc, pool, st, name):
    """4 slot weights [P,NCH] in patch-slot order [x0r0, x0r1, x1r0, x1r1]."""
    _, ix, iy, x0, y0, xs, ys = st
    fx1 = pool.tile([P, NCH], f32, tag=f"{name}_fx1")
    nc.vector.tensor_tensor(out=fx1[:], in0=ix[:], in1=x0[:], op=Alu.subtract)
    fy1 = pool.tile([P, NCH], f32, tag=f"{name}_fy1")
    nc.vector.tensor_tensor(out=fy1[:], in0=iy[:], in1=y0[:], op=Alu.subtract)
    a0, a1 = _axis_slot_weights(nc, pool, x0, fx1, xs, f"{name}_x")
    b0, b1 = _axis_slot_weights(nc, pool, y0, fy1, ys, f"{name}_y")
    ws = []
    for wi, (ca, cb) in enumerate(((a0, b0), (a0, b1), (a1, b0), (a1, b1))):
        wt = pool.tile([P, NCH], f32, tag=f"{name}_w{wi}")
        nc.vector.tensor_tensor(out=wt[:], in0=ca[:], in1=cb[:], op=Alu.mult)
        ws.append(wt)
    return ws


def build_module():
    nc = bacc.Bacc("TRN2", target_bir_lowering=False, debug=False, num_devices=8)

    feat2 = nc.dram_tensor("feat2", [2 * NBLK, 2 * D], f32, kind="ExternalInput")
    mask = nc.dram_tensor("mask", [2, H, W], f32, kind="ExternalInput")
    px_in = nc.dram_tensor("px", [P, NCH], f32, kind="ExternalInput")
    py_in = nc.dram_tensor("py", [P, NCH], f32, kind="ExternalInput")
    wv_in = nc.dram_tensor("withinv", [P, 1, 16], f32, kind="ExternalInput")
    jc_in = nc.dram_tensor("sixteen_minus_j", [P, 1, 16], f32, kind="ExternalInput")
    out_t = nc.dram_tensor("out_t", [NPTS, D], f32, kind="ExternalOutput")
    out_s = nc.dram_tensor("out_s", [NPTS, D], f32, kind="ExternalOutput")
    out_p = nc.dram_tensor("out_p", [P, NCH, 2], f32, kind="ExternalOutput")
    mp = nc.dram_tensor("mp", [MP_FLAT, 1], f32)

    mp_h = mp[:].tensor

    with tile.TileContext(nc) as tc:
        with (
            tc.tile_pool(name="consts", bufs=1) as cpool,
            tc.tile_pool(name="scal", bufs=1) as sp,
            tc.tile_pool(name="maskp", bufs=1) as mkp,
            tc.tile_pool(name="span", bufs=6) as spanp,
            tc.tile_pool(name="patch", bufs=16) as patp,
            tc.tile_pool(name="fout", bufs=8) as fp,
        ):
            # ---- points + consts first (unblock DVE + gpsimd quickly) ----
            x = sp.tile([P, NCH], f32, tag="x")
            nc.sync.dma_start(out=x[:], in_=px_in[:])
            y = sp.tile([P, NCH], f32, tag="y")
            nc.sync.dma_start(out=y[:], in_=py_in[:])
            wv = cpool.tile([P, 1, 16], f32)
            nc.sync.dma_start(out=wv[:], in_=wv_in[:])
            jc = cpool.tile([P, 1, 16], f32)
            nc.sync.dma_start(out=jc[:], in_=jc_in[:])

            # ---- source sites -> source gathers start ASAP ----
            st_s = _bilinear_sites(nc, sp, x, y, "src")
            sidx_s = st_s[0]

            def gather_chunk(c, sidx_i):
                patch = patp.tile([P, 4 * D], f32, tag="patch")
                nc.gpsimd.indirect_dma_start(
                    out=patch[:], out_offset=None, in_=feat2[:],
                    in_offset=bass.IndirectOffsetOnAxis(ap=sidx_i[:, c, 0:1], axis=0),
                )
                return patch

            def sum_chunks(patches, ws, out_dram, g):
                # 4-chunk group: weighted patch sums then one 512KiB store.
                # exact reference association: ((v00+v01)+v10)+v11 with
                # patch slots [x0r0, x0r1, x1r0, x1r1]
                f4 = fp.tile([P, 4, D], f32, tag="f4")
                for k in range(4):
                    c = 4 * g + k
                    patch = patches[c]
                    nc.scalar.mul(
                        out=f4[:, k, :], in_=patch[:, 0:D], mul=ws[0][:, c : c + 1]
                    )
                    for wi, off in ((2, 2 * D), (1, D), (3, 3 * D)):
                        nc.vector.scalar_tensor_tensor(
                            out=f4[:, k, :], in0=patch[:, off : off + D],
                            scalar=ws[wi][:, c : c + 1], in1=f4[:, k, :],
                            op0=Alu.mult, op1=Alu.add,
                        )
                nc.sync.dma_start(
                    out=out_dram[:].rearrange("(g p) d -> g p d", p=4 * P)[g]
                    .rearrange("(c p) d -> p c d", p=P),
                    in_=f4[:],
                )

            ws_s = _bilinear_weights(nc, sp, st_s, "src")
            src_patches = [gather_chunk(c, sidx_s) for c in range(NCH)]

            # ---- padded mask plane (scalar-engine DMAs) ----
            negt = cpool.tile([P, 2115], f32)
            nc.vector.memset(negt[:], NEG)
            mp_flat_ap = bass.AP(mp_h, 0, [[2115, P], [1, 2115]])
            fill = nc.scalar.dma_start(out=mp_flat_ap, in_=negt[:])
            mload = mkp.tile([P, 2, 4, W], f32)
            mask_ap = bass.AP(mask[:].tensor, 0, [[W, P], [H * W, 2], [P * W, 4], [1, W]])
            nc.scalar.dma_start(out=mload[:], in_=mask_ap)
            msum = mkp.tile([P, 4, W], f32)
            nc.vector.tensor_tensor(
                out=msum[:], in0=mload[:, 0], in1=mload[:, 1], op=Alu.add
            )
            mp_int_ap = bass.AP(mp_h, 2 * S + 2, [[S, P], [P * S, 4], [1, W]])
            intw = nc.scalar.dma_start(out=mp_int_ap, in_=msum[:])
            tile.add_dep_helper(intw.ins, fill.ins, reason="mp fill before interior")

            # ---- snap window bases ----
            xc = sp.tile([P, NCH], f32, tag="xc")
            nc.vector.tensor_scalar(
                out=xc[:], in0=x[:], scalar1=0.0, scalar2=float(H),
                op0=Alu.max, op1=Alu.min,
            )
            yc = sp.tile([P, NCH], f32, tag="yc")
            nc.vector.tensor_scalar(
                out=yc[:], in0=y[:], scalar1=0.0, scalar2=float(W),
                op0=Alu.max, op1=Alu.min,
            )
            xi = _exact_floor(nc, sp, xc, "xi")
            yi = _exact_floor(nc, sp, yc, "yi")
            wbase = sp.tile([P, NCH, 2], f32, tag="wbase")
            nc.vector.scalar_tensor_tensor(
                out=wbase[:, :, 0], in0=xi[:], scalar=float(S), in1=yi[:],
                op0=Alu.mult, op1=Alu.add,
            )
            nc.vector.tensor_scalar(
                out=wbase[:, :, 1], in0=wbase[:, :, 0], scalar1=float(2 * S),
                scalar2=None, op0=Alu.add,
            )
            wbase_i = sp.tile([P, NCH, 2], i32, tag="wbasei")
            nc.vector.tensor_copy(wbase_i[:], wbase[:])

            # ---- source sums first on DVE (their data lands earliest) ----
            for g in range(NCH // 4):
                sum_chunks(src_patches, ws_s, out_s, g)

            # ---- window gathers: one 4-row span per chunk ----
            mwin = sp.tile([P, NCH, 16], f32, tag="mwin")
            for c in range(NCH):
                span = spanp.tile([P, SPAN4], f32, tag="span")
                g = nc.gpsimd.indirect_dma_start(
                    out=span[:], out_offset=None, in_=mp[:],
                    in_offset=bass.IndirectOffsetOnAxis(ap=wbase_i[:, c, 0:1], axis=0),
                )
                tile.add_dep_helper(g.ins, intw.ins, reason="mp before win gather")
                src = span[:, 0:4]
                src2 = bass.AP(src.tensor, src.offset, [src.ap[0], [S, 4], [1, 4]])
                dst = mwin[:, c, :]
                dst2 = bass.AP(dst.tensor, dst.offset, [dst.ap[0], [4, 4], [1, 4]])
                nc.vector.tensor_copy(out=dst2, in_=src2)

            # ---- disk-masked first-index argmax ----
            swin = sp.tile([P, NCH, 16], f32, tag="swin")
            nc.vector.tensor_tensor(
                out=swin[:], in0=mwin[:], in1=wv[:].to_broadcast([P, NCH, 16]), op=Alu.mult
            )
            mx = sp.tile([P, NCH, 1], f32, tag="mx")
            nc.vector.tensor_reduce(out=mx[:], in_=swin[:], axis=Ax.X, op=Alu.max)
            eq = sp.tile([P, NCH, 16], f32, tag="eq")
            nc.vector.tensor_tensor(
                out=eq[:], in0=swin[:], in1=mx[:].to_broadcast([P, NCH, 16]), op=Alu.is_ge
            )
            cand = sp.tile([P, NCH, 16], f32, tag="cand")
            nc.vector.tensor_tensor(
                out=cand[:], in0=eq[:], in1=jc[:].to_broadcast([P, NCH, 16]), op=Alu.mult
            )
            nc.vector.tensor_scalar(
                out=cand[:], in0=cand[:], scalar1=-1.0, scalar2=16.0,
                op0=Alu.mult, op1=Alu.add,
            )
            kf = sp.tile([P, NCH], f32, tag="kf")
            nc.vector.tensor_reduce(out=kf[:], in_=cand[:], axis=Ax.X, op=Alu.min)

            ka = sp.tile([P, NCH], f32, tag="ka")
            nc.vector.tensor_scalar(out=ka[:], in0=kf[:], scalar1=4.0, scalar2=None, op0=Alu.is_ge)
            kt = sp.tile([P, NCH], f32, tag="kt")
            for thr in (8.0, 12.0):
                nc.vector.tensor_scalar(out=kt[:], in0=kf[:], scalar1=thr, scalar2=None, op0=Alu.is_ge)
                nc.vector.tensor_tensor(out=ka[:], in0=ka[:], in1=kt[:], op=Alu.add)
            kb = sp.tile([P, NCH], f32, tag="kb")
            nc.vector.scalar_tensor_tensor(
                out=kb[:], in0=ka[:], scalar=-4.0, in1=kf[:], op0=Alu.mult, op1=Alu.add
            )
            xn = sp.tile([P, NCH], f32, tag="xn")
            nc.vector.scalar_tensor_tensor(
                out=xn[:], in0=ka[:], scalar=-2.0, in1=xi[:], op0=Alu.add, op1=Alu.add
            )
            yn = sp.tile([P, NCH], f32, tag="yn")
            nc.vector.scalar_tensor_tensor(
                out=yn[:], in0=kb[:], scalar=-2.0, in1=yi[:], op0=Alu.add, op1=Alu.add
            )
            ex = sp.tile([P, NCH], f32, tag="ex")
            nc.vector.tensor_scalar(out=ex[:], in0=x[:], scalar1=0.0, scalar2=None, op0=Alu.is_equal)
            ey = sp.tile([P, NCH], f32, tag="ey")
            nc.vector.tensor_scalar(out=ey[:], in0=y[:], scalar1=0.0, scalar2=None, op0=Alu.is_equal)
            keep = sp.tile([P, NCH], f32, tag="keep")
            nc.vector.tensor_tensor(out=keep[:], in0=ex[:], in1=ey[:], op=Alu.mult)
            keep_i = sp.tile([P, NCH], i32, tag="keepi")
            nc.vector.tensor_copy(keep_i[:], keep[:])
            ptx = sp.tile([P, NCH], f32, tag="ptx")
            nc.vector.tensor_copy(ptx[:], xn[:])
            nc.vector.copy_predicated(ptx[:], keep_i[:], x[:])
            pty = sp.tile([P, NCH], f32, tag="pty")
            nc.vector.tensor_copy(pty[:], yn[:])
            nc.vector.copy_predicated(pty[:], keep_i[:], y[:])

            po = sp.tile([P, NCH, 2], f32, tag="po")
            nc.vector.tensor_copy(po[:, :, 0], ptx[:])
            nc.vector.tensor_copy(po[:, :, 1], pty[:])
            nc.sync.dma_start(out=out_p[:], in_=po[:])

            # ---- target sites/weights, then gathers ----
            st_t = _bilinear_sites(nc, sp, ptx, pty, "tgt")
            ws_t = _bilinear_weights(nc, sp, st_t, "tgt")
            tgt_patches = [gather_chunk(c, st_t[0]) for c in range(NCH)]

            # ---- target sums + grouped stores ----
            for g in range(NCH // 4):
                sum_chunks(tgt_patches, ws_t, out_t, g)

    nc.finalize()
    return nc


def _host_consts():
    t = np.arange(16)
    a = t // 4 - 2
    b = t % 4 - 2
    within = ((a * a + b * b) <= 4).astype(np.float32)
    wv = np.ascontiguousarray(np.tile(within[None, None, :], (P, 1, 1)))
    jc = np.ascontiguousarray(
        np.tile((16.0 - t).astype(np.float32)[None, None, :], (P, 1, 1))
    )
    return wv, jc


def _pair_table(fmap_b):
    """[D,H,W] -> [2*NBLK, 2*D] patch table (even + odd row pairings)."""
    ft = np.ascontiguousarray(fmap_b.transpose(1, 2, 0))  # [H, W, D]
    tab = np.zeros((2, H // 2, W, 2, D), dtype=np.float32)
    tab[0] = ft.reshape(H // 2, 2, W, D).transpose(0, 2, 1, 3)
    tab[1, : H // 2 - 1] = ft[1 : H - 1].reshape(H // 2 - 1, 2, W, D).transpose(
        0, 2, 1, 3
    )
    return tab.reshape(2 * NBLK, 2 * D)


def _shard_inputs(feature_maps, sample_points, mask_scores):
    wv, jc = _host_consts()
    tables = {}
    masks = {}
    in_maps = []
    for core in range(8):
        b, half = divmod(core, 2)
        if b not in tables:
            tables[b] = _pair_table(feature_maps[b])
            masks[b] = np.ascontiguousarray(mask_scores[b])
        pts = sample_points[b, half * NPTS : (half + 1) * NPTS]
        # device layout [p, c]: point n = c*128 + p
        px = np.ascontiguousarray(pts[:, 0].reshape(NCH, P).T)
        py = np.ascontiguousarray(pts[:, 1].reshape(NCH, P).T)
        in_maps.append(
            {
                "feat2": tables[b],
                "mask": masks[b],
                "px": px,
                "py": py,
                "withinv": wv,
                "sixteen_minus_j": jc,
            }
        )
    return in_maps


def run(feature_maps, sample_points, mask_scores, trace=False):
    if "nc" not in _CACHE:
        _CACHE["nc"] = build_module()
    nc = _CACHE["nc"]
    in_maps = _shard_inputs(feature_maps, sample_points, mask_scores)
    res = run_bass_kernel_spmd(nc, in_maps, core_ids=list(range(8)), trace=trace)
    B, N = 4, 4096
    tgt = np.empty((B, N, D), np.float32)
    src = np.empty((B, N, D), np.float32)
    pt = np.empty((B, N, 2), np.float32)
    for core in range(8):
        b, half = divmod(core, 2)
        sl = slice(half * NPTS, (half + 1) * NPTS)
        r = res.results[core]
        tgt[b, sl] = r["out_t"]
        src[b, sl] = r["out_s"]
        pt[b, sl] = r["out_p"].transpose(1, 0, 2).reshape(NPTS, 2)
    return (tgt, pt, src), res


def kernel(feature_maps, sample_points, mask_scores):
    outs, _ = run(feature_maps, sample_points, mask_scores)
    return outs
